# revision 9
# baseline (speedup 1.0000x reference)
"""Trainium2 Bass kernel for MyMultiAttentionLayer.

Model (reference):
    q = einsum('bsd,hpd->bhsp', x, q_w) + q_b      (same for k, v)
    scores = q @ k^T / sqrt(P)                      [B,H,S,S]
    attn = softmax(scores, axis=2)                  # softmax over the QUERY axis
    ctx = einsum('bhqk,bhkp->bqhp', attn, v)
    out = concat(ctx) @ l_w.T + l_b                 [B,S,NUM_OUT]

Shapes: B=2, S=2048, D=1024, H=16, P=64, NUM_OUT=1024.

Sharding: 8 cores = 2 batches x 4 head-groups (4 heads each).  Each core
computes its batch's attention for its 4 heads plus the partial output
projection over its 256 features; the host sums the 4 partials per batch
(all-reduce equivalent) and adds l_b.

Key algebraic trick: softmax is over the query axis, so the normalizer
Z[k] = sum_q exp(s[q,k]) depends only on k.  ctx = sum_k e[q,k]*(v[k,:]/Z[k]),
i.e. the normalization is folded into the tiny v rows (64 wide) instead of
the 2048-wide attention matrix.  exp() runs on the scalar engine over
[128,1024] PSUM blocks with a fused free-axis accumulate producing Z.

Precision: projections / scores / output projection in float32r (fp32 data,
fast PE mode); exp outputs and the normalized v in bf16 (feeds only the
attn @ v matmul; adds ~2e-3 relative error).

The PE executes its instruction stream in order, so the attention loop is
software-pipelined at emission time: ctx matmuls of tile t-2 are emitted
after the scores matmuls of tile t, keeping the PE busy while the scalar
engine works on exp(t-1)/exp(t) — otherwise the PE idles every iteration,
the HAM activity monitor re-throttles it to 1.2 GHz, and every matmul
doubles in cost (measured: 410us with 3/4 of the run at K=4/8).

Per-core layouts (transposes are done host-side when staging inputs):
  xt  [D,S]   = x[b].T                    (contraction dim d on partitions)
  qwT [D,4P]  (d, (h,p))                  kwT same, vwT same
  qb  [4P,1]  kb [4P,1]  vb [1,4P]
  lwT [4P,NUM_OUT] = l_w[:, feat_slice].T
  out [S,NUM_OUT] partial (no l_b)
"""

import numpy as np

import concourse.bass as bass
import concourse.tile as tile
from concourse import bacc, mybir
from concourse.bass_utils import run_bass_kernel_spmd

B, S, D = 2, 2048, 1024
H, P = 16, 64
NUM_OUT = 1024
N_CORES = 8
HPC = 4                 # heads per core
PAIRS = 2               # head pairs per core (2 heads x 64 = 128 partitions)
DT = D // 128           # 8 d-tiles
ST = S // 128           # 16 s-tiles
SC = S // 512           # 4 s-chunks of 512
NC_CH = NUM_OUT // 512  # 2 output chunks
PIPE = 2                # ctx matmuls trail scores by this many ki-tiles

F32 = mybir.dt.float32
F32R = mybir.dt.float32r
BF16 = mybir.dt.bfloat16
EXP = mybir.ActivationFunctionType.Exp
AX = mybir.AxisListType.X


def build_nc():
    nc = bacc.Bacc("TRN2", target_bir_lowering=False, debug=False,
                   num_devices=N_CORES)

    xt_d = nc.dram_tensor("xt", [D, S], F32R, kind="ExternalInput")
    qwT_d = nc.dram_tensor("qwT", [D, HPC * P], F32R, kind="ExternalInput")
    kwT_d = nc.dram_tensor("kwT", [D, HPC * P], F32R, kind="ExternalInput")
    vwT_d = nc.dram_tensor("vwT", [D, HPC * P], F32R, kind="ExternalInput")
    qb_d = nc.dram_tensor("qb", [HPC * P, 1], F32, kind="ExternalInput")
    kb_d = nc.dram_tensor("kb", [HPC * P, 1], F32, kind="ExternalInput")
    vb_d = nc.dram_tensor("vb", [1, HPC * P], F32R, kind="ExternalInput")
    lwT_d = nc.dram_tensor("lwT", [HPC * P, NUM_OUT], F32R, kind="ExternalInput")
    ones_d = nc.dram_tensor("ones", [1, 128], F32R, kind="ExternalInput")
    out_d = nc.dram_tensor("out", [S, NUM_OUT], F32, kind="ExternalOutput")

    with tile.TileContext(nc) as tc:
        with (
            tc.tile_pool(name="qk", bufs=4) as p_qk,
            tc.tile_pool(name="vv", bufs=ST) as p_v,
            tc.tile_pool(name="cst", bufs=1) as p_c,
            tc.tile_pool(name="zz", bufs=4) as p_z,
            tc.tile_pool(name="et", bufs=2 * (PIPE + 1)) as p_et,
            tc.tile_pool(name="cc", bufs=HPC) as p_cc,
            tc.tile_pool(name="ob", bufs=2) as p_ob,
            tc.tile_pool(name="xt", bufs=DT) as p_xt,
            tc.tile_pool(name="wst", bufs=2 * DT) as p_w,
            tc.tile_pool(name="mm", bufs=2, space=bass.MemorySpace.PSUM) as p_mm,
            tc.tile_pool(name="cx", bufs=4, space=bass.MemorySpace.PSUM) as p_cx,
        ):
            # ---- stage inputs, in the order the PE needs them ----
            qb_t, kb_t = [], []
            for pr in range(PAIRS):
                t = p_c.tile([128, 1], F32, name=f"qb{pr}", tag=f"qb{pr}")
                nc.sync.dma_start(t[:], qb_d[pr * 128:(pr + 1) * 128, :])
                qb_t.append(t)
                t = p_c.tile([128, 1], F32, name=f"kb{pr}", tag=f"kb{pr}")
                nc.sync.dma_start(t[:], kb_d[pr * 128:(pr + 1) * 128, :])
                kb_t.append(t)
            vb_t = p_c.tile([1, HPC * P], F32R, name="vb", tag="vb")
            nc.sync.dma_start(vb_t[:], vb_d[:, :])
            ones = p_c.tile([1, 128], F32R, name="ones", tag="ones")
            nc.sync.dma_start(ones[:], ones_d[:, :])

            xt, wq, wk, wv = [], [], [], []
            for d in range(DT):
                t = p_w.tile([128, HPC * P], F32R, name=f"qw{d}", tag="w")
                nc.sync.dma_start(t[:], qwT_d[d * 128:(d + 1) * 128, :])
                wq.append(t)
                t = p_w.tile([128, HPC * P], F32R, name=f"kw{d}", tag="w")
                nc.sync.dma_start(t[:], kwT_d[d * 128:(d + 1) * 128, :])
                wk.append(t)
                t = p_xt.tile([128, S], F32R, name=f"xt{d}", tag="xt")
                nc.sync.dma_start(t[:], xt_d[d * 128:(d + 1) * 128, :])
                xt.append(t)
            for d in range(DT):
                t = p_w.tile([128, HPC * P], F32R, name=f"vw{d}", tag="w")
                nc.sync.dma_start(t[:], vwT_d[d * 128:(d + 1) * 128, :])
                wv.append(t)
            lw_t = []
            for h in range(HPC):
                t = p_c.tile([64, NUM_OUT], F32R, name=f"lw{h}", tag=f"lw{h}")
                nc.sync.dma_start(t[:], lwT_d[h * 64:(h + 1) * 64, :])
                lw_t.append(t)

            # ---- q/k projections: qT/kT [128=(2 heads x P), S] per pair ----
            # out[p_hp, s] = sum_d wT[d, p_hp] * xt[d, s]
            qkT = {"q": [], "k": []}
            for nm, wts, bias in (("q", wq, qb_t), ("k", wk, kb_t)):
                for pr in range(PAIRS):
                    ps = [p_mm.tile([128, 1024], F32, name=f"ps_{nm}{pr}{i}",
                                    tag="mm") for i in range(2)]
                    halves = [(ps[0], 0), (ps[0], 512), (ps[1], 0),
                              (ps[1], 512)]
                    for d in range(DT):
                        lhsT = wts[d][:, pr * 128:(pr + 1) * 128]
                        for c, (pt, off) in enumerate(halves):
                            nc.tensor.matmul(
                                pt[:, off:off + 512], lhsT,
                                xt[d][:, c * 512:(c + 1) * 512],
                                start=(d == 0), stop=(d == DT - 1))
                    dst = p_qk.tile([128, S], F32R, name=f"{nm}T{pr}",
                                    tag="qk")
                    for c, (pt, off) in enumerate(halves):
                        nc.vector.tensor_scalar_add(
                            dst[:, c * 512:(c + 1) * 512],
                            pt[:, off:off + 512], bias[pr][:])
                    qkT[nm].append(dst)

            # ---- v projection: v [128=s, 4P=(h,p)] per s-tile (+ bias) ----
            # out[s, hp] = sum_d xt[d, s] * vwT[d, hp]  (+ ones^T @ vb)
            v_t = []
            for st in range(ST):
                ps = p_mm.tile([128, 1024], F32, name=f"ps_v{st}", tag="mm")
                for d in range(DT):
                    nc.tensor.matmul(
                        ps[:, :HPC * P],
                        xt[d][:, st * 128:(st + 1) * 128],
                        wv[d][:],
                        start=(d == 0), stop=False)
                nc.tensor.matmul(ps[:, :HPC * P], ones[:], vb_t[:],
                                 start=False, stop=True)
                dst = p_v.tile([128, HPC * P], F32R, name=f"v{st}", tag="v")
                nc.vector.tensor_copy(dst[:], ps[:, :HPC * P])
                v_t.append(dst)

            # ---- attention, one head at a time, software-pipelined ----
            # scoresT[k_i, q_i] = sum_p kT[p, k_i] * qT[p, q_i]   (K=64)
            # eT = exp(scoresT/8) with fused Z = sum_q (scalar engine)
            # v' = v * (1/Z) per k row; ctxT[p, q_i] += vs^T-stationary @ eT
            ctxT = []
            for h in range(HPC):
                pr, off = divmod(h, 2)
                off *= 64
                kT, qT = qkT["k"][pr], qkT["q"][pr]
                cps = [p_cx.tile([64, 512], F32, name=f"cx{h}{c}", tag="cx")
                       for c in range(SC)]
                pending = []

                def flush_ctx():
                    t0, ets0, vs0 = pending.pop(0)
                    for c in range(SC):
                        nc.tensor.matmul(
                            cps[c][:], vs0,
                            ets0[c // 2][:, (c % 2) * 512:(c % 2) * 512 + 512],
                            start=(t0 == 0), stop=(t0 == ST - 1))

                for t in range(ST):
                    pa = p_mm.tile([128, 1024], F32, name=f"ps_s{h}{t}a",
                                   tag="mm")
                    pb = p_mm.tile([128, 1024], F32, name=f"ps_s{h}{t}b",
                                   tag="mm")
                    lhsT = kT[off:off + 64, t * 128:(t + 1) * 128]
                    for c, (pt, o2) in enumerate(
                            ((pa, 0), (pa, 512), (pb, 0), (pb, 512))):
                        nc.tensor.matmul(
                            pt[:, o2:o2 + 512], lhsT,
                            qT[off:off + 64, c * 512:(c + 1) * 512],
                            start=True, stop=True)
                    zp = p_z.tile([128, 2], F32, name=f"zp{h}{t}", tag="zp")
                    et0 = p_et.tile([128, 1024], BF16, name=f"et{h}{t}a",
                                    tag="et")
                    et1 = p_et.tile([128, 1024], BF16, name=f"et{h}{t}b",
                                    tag="et")
                    nc.scalar.activation(et0[:], pa[:], EXP, scale=0.125,
                                         accum_out=zp[:, 0:1])
                    nc.scalar.activation(et1[:], pb[:], EXP, scale=0.125,
                                         accum_out=zp[:, 1:2])
                    z = p_z.tile([128, 1], F32, name=f"z{h}{t}", tag="z")
                    nc.vector.reduce_sum(z[:], zp[:], axis=AX)
                    zr = p_z.tile([128, 1], F32, name=f"zr{h}{t}", tag="zr")
                    nc.vector.reciprocal(zr[:], z[:])
                    vs = p_z.tile([128, 64], BF16, name=f"vs{h}{t}", tag="vs")
                    nc.vector.tensor_scalar_mul(
                        vs[:], v_t[t][:, h * 64:(h + 1) * 64], zr[:])
                    pending.append((t, (et0, et1), vs))
                    if len(pending) > PIPE:
                        flush_ctx()
                while pending:
                    flush_ctx()

                dst = p_cc.tile([64, S], F32R, name=f"ctxT{h}", tag="cc")
                for c in range(SC):
                    nc.vector.tensor_copy(dst[:, c * 512:(c + 1) * 512],
                                          cps[c][:])
                ctxT.append(dst)

            # ---- output projection ----
            # out[s, n] = sum_h sum_p ctxT_h[p, s] * lwT_h[p, n]
            for st in range(ST):
                ob = p_ob.tile([128, NUM_OUT], F32, name=f"ob{st}", tag="ob")
                po = p_mm.tile([128, 1024], F32, name=f"ps_o{st}", tag="mm")
                for ncn in range(NC_CH):
                    o2 = ncn * 512
                    for h in range(HPC):
                        nc.tensor.matmul(
                            po[:, o2:o2 + 512],
                            ctxT[h][:, st * 128:(st + 1) * 128],
                            lw_t[h][:, ncn * 512:(ncn + 1) * 512],
                            start=(h == 0), stop=(h == HPC - 1))
                    nc.vector.tensor_copy(ob[:, o2:o2 + 512],
                                          po[:, o2:o2 + 512])
                nc.sync.dma_start(out_d[st * 128:(st + 1) * 128, :], ob[:])

    nc.compile()
    return nc


_NC_CACHE = None


def _get_nc():
    global _NC_CACHE
    if _NC_CACHE is None:
        _NC_CACHE = build_nc()
    return _NC_CACHE


def _prep_in_maps(x, q_w, q_b, k_w, k_b, v_w, v_b, l_w):
    """Host-side sharding: per-core input dict (core = b*4 + g)."""
    in_maps = []
    xts = [np.ascontiguousarray(x[b].T) for b in range(B)]
    ones = np.ones((1, 128), dtype=np.float32)
    for b in range(B):
        for g in range(4):
            hs = slice(g * HPC, (g + 1) * HPC)
            f0, f1 = g * HPC * P, (g + 1) * HPC * P
            in_maps.append({
                "xt": xts[b],
                "qwT": np.ascontiguousarray(
                    q_w[hs].transpose(2, 0, 1).reshape(D, HPC * P)),
                "kwT": np.ascontiguousarray(
                    k_w[hs].transpose(2, 0, 1).reshape(D, HPC * P)),
                "vwT": np.ascontiguousarray(
                    v_w[hs].transpose(2, 0, 1).reshape(D, HPC * P)),
                "qb": np.ascontiguousarray(q_b[hs].reshape(HPC * P, 1)),
                "kb": np.ascontiguousarray(k_b[hs].reshape(HPC * P, 1)),
                "vb": np.ascontiguousarray(v_b[hs].reshape(1, HPC * P)),
                "lwT": np.ascontiguousarray(l_w[:, f0:f1].T),
                "ones": ones,
            })
    return in_maps


def _run(inputs, trace=False):
    f32 = lambda a: np.asarray(a, dtype=np.float32)
    x = f32(inputs["x"])
    l_b = f32(inputs["l_b"])
    in_maps = _prep_in_maps(
        x, f32(inputs["q_w"]), f32(inputs["q_b"]), f32(inputs["k_w"]),
        f32(inputs["k_b"]), f32(inputs["v_w"]), f32(inputs["v_b"]),
        f32(inputs["l_w"]))
    nc = _get_nc()
    res = run_bass_kernel_spmd(nc, in_maps, list(range(N_CORES)), trace=trace)
    out = np.empty((B, S, NUM_OUT), dtype=np.float32)
    for b in range(B):
        acc = res.results[b * 4]["out"].astype(np.float32)
        for g in range(1, 4):
            acc = acc + res.results[b * 4 + g]["out"]
        out[b] = acc + l_b
    return out, res


def kernel(**inputs):
    out, _ = _run(inputs, trace=False)
    return out


# revision 12
# speedup vs baseline: 1.0207x; 1.0207x over previous
"""Trainium2 Bass kernel for MyMultiAttentionLayer.

Model (reference):
    q = einsum('bsd,hpd->bhsp', x, q_w) + q_b      (same for k, v)
    scores = q @ k^T / sqrt(P)                      [B,H,S,S]
    attn = softmax(scores, axis=2)                  # softmax over the QUERY axis
    ctx = einsum('bhqk,bhkp->bqhp', attn, v)
    out = concat(ctx) @ l_w.T + l_b                 [B,S,NUM_OUT]

Shapes: B=2, S=2048, D=1024, H=16, P=64, NUM_OUT=1024.

Sharding: 8 cores = 2 batches x 4 head-groups (4 heads each).  Each core
computes its batch's attention for its 4 heads plus the partial output
projection over its 256 features; the host sums the 4 partials per batch
(all-reduce equivalent) and adds l_b.

Softmax is over the query axis, so the normalizer Z[k] = sum_q exp(s[q,k])
depends only on k: ctx = sum_k e[q,k]*(v[k,:]/Z[k]) — the normalization is
folded into the 64-wide v rows instead of the 2048-wide attention matrix.
exp() runs on the scalar engine over [128,1024] PSUM blocks with a fused
free-axis accumulate producing Z; cost (N+352)/1.2 ns makes the scalar
engine the attention-phase throughput limit at ~2.9us per ki-tile.

The PE executes its stream in order and the HAM clock gate drops it to
1.2 GHz after ~3.4us of idle, so the schedule is built to keep the PE
continuously busy:
  * the pair-0 q/k projections run first (DMA-paced),
  * the v projection and pair-1 q/k projections are chopped into
    transient-PSUM units (8-9 matmuls each) and interleaved into the
    attention loop of heads 0/1 as filler the PE can chew while the
    scalar engine works off exp backlog,
  * ctx accumulates in PSUM in groups of 4 ki-tiles, drained to an fp32
    SBUF accumulator by the vector engine (frees 2 PSUM banks for the
    filler units),
  * attention scores for ki-tile t are emitted before the ctx matmuls of
    the previous group (software pipelining; the PE never waits on the
    exp of the tile it just produced).

Precision: projections / scores / output projection in float32r (fp32
data, fast PE mode); exp outputs and v/Z in bf16 (feed only the attn @ v
matmul; ~2e-3 relative error total).

Per-core layouts (transposes are done host-side when staging inputs):
  xt  [D,S]   = x[b].T                    (contraction dim d on partitions)
  qwT [D,4P]  (d, (h,p))                  kwT same, vwT same
  qb  [4P,1]  kb [4P,1]  vb [1,4P]
  lwT [4P,NUM_OUT] = l_w[:, feat_slice].T
  out [S,NUM_OUT] partial (no l_b)
"""

import numpy as np

import concourse.bass as bass
import concourse.tile as tile
from concourse import bacc, mybir
from concourse.bass_utils import run_bass_kernel_spmd

B, S, D = 2, 2048, 1024
H, P = 16, 64
NUM_OUT = 1024
N_CORES = 8
HPC = 4                 # heads per core
PAIRS = 2               # head pairs per core (2 heads x 64 = 128 partitions)
DT = D // 128           # 8 d-tiles
ST = S // 128           # 16 s-tiles
SC = S // 512           # 4 s-chunks of 512
NC_CH = NUM_OUT // 512  # 2 output chunks
GRP = 4                 # ki-tiles per ctx PSUM accumulation group

F32 = mybir.dt.float32
F32R = mybir.dt.float32r
BF16 = mybir.dt.bfloat16
EXP = mybir.ActivationFunctionType.Exp
AX = mybir.AxisListType.X


def build_nc():
    nc = bacc.Bacc("TRN2", target_bir_lowering=False, debug=False,
                   num_devices=N_CORES)

    xt_d = nc.dram_tensor("xt", [D, S], F32R, kind="ExternalInput")
    qwT_d = nc.dram_tensor("qwT", [D, HPC * P], F32R, kind="ExternalInput")
    kwT_d = nc.dram_tensor("kwT", [D, HPC * P], F32R, kind="ExternalInput")
    vwT_d = nc.dram_tensor("vwT", [D, HPC * P], F32R, kind="ExternalInput")
    qb_d = nc.dram_tensor("qb", [HPC * P, 1], F32, kind="ExternalInput")
    kb_d = nc.dram_tensor("kb", [HPC * P, 1], F32, kind="ExternalInput")
    vb_d = nc.dram_tensor("vb", [1, HPC * P], F32R, kind="ExternalInput")
    lwT_d = nc.dram_tensor("lwT", [HPC * P, NUM_OUT], F32R, kind="ExternalInput")
    ones_d = nc.dram_tensor("ones", [1, 128], F32R, kind="ExternalInput")
    out_d = nc.dram_tensor("out", [S, NUM_OUT], F32, kind="ExternalOutput")

    with tile.TileContext(nc) as tc:
        with (
            tc.tile_pool(name="qk", bufs=4) as p_qk,
            tc.tile_pool(name="vv", bufs=ST) as p_v,
            tc.tile_pool(name="cst", bufs=1) as p_c,
            tc.tile_pool(name="zz", bufs=6) as p_z,
            tc.tile_pool(name="et", bufs=8) as p_et,
            tc.tile_pool(name="cc", bufs=HPC) as p_cc,
            tc.tile_pool(name="ob", bufs=1) as p_ob,
            tc.tile_pool(name="xt", bufs=DT) as p_xt,
            tc.tile_pool(name="wst", bufs=3 * DT) as p_w,
            tc.tile_pool(name="mm", bufs=2, space=bass.MemorySpace.PSUM) as p_mm,
            tc.tile_pool(name="cx", bufs=2, space=bass.MemorySpace.PSUM) as p_cx,
            tc.tile_pool(name="pf", bufs=2, space=bass.MemorySpace.PSUM) as p_pf,
        ):
            # ---- stage inputs, in the order the PE needs them ----
            qb_t, kb_t = [], []
            for pr in range(PAIRS):
                t = p_c.tile([128, 1], F32, name=f"qb{pr}", tag=f"qb{pr}")
                nc.sync.dma_start(t[:], qb_d[pr * 128:(pr + 1) * 128, :])
                qb_t.append(t)
                t = p_c.tile([128, 1], F32, name=f"kb{pr}", tag=f"kb{pr}")
                nc.sync.dma_start(t[:], kb_d[pr * 128:(pr + 1) * 128, :])
                kb_t.append(t)
            vb_t = p_c.tile([1, HPC * P], F32R, name="vb", tag="vb")
            nc.sync.dma_start(vb_t[:], vb_d[:, :])
            ones = p_c.tile([1, 128], F32R, name="ones", tag="ones")
            nc.sync.dma_start(ones[:], ones_d[:, :])

            xt, wq, wk, wv = [], [], [], []
            for d in range(DT):
                t = p_w.tile([128, HPC * P], F32R, name=f"qw{d}", tag="w")
                nc.sync.dma_start(t[:], qwT_d[d * 128:(d + 1) * 128, :])
                wq.append(t)
                t = p_w.tile([128, HPC * P], F32R, name=f"kw{d}", tag="w")
                nc.sync.dma_start(t[:], kwT_d[d * 128:(d + 1) * 128, :])
                wk.append(t)
                t = p_xt.tile([128, S], F32R, name=f"xt{d}", tag="xt")
                nc.sync.dma_start(t[:], xt_d[d * 128:(d + 1) * 128, :])
                xt.append(t)
            for d in range(DT):
                t = p_w.tile([128, HPC * P], F32R, name=f"vw{d}", tag="w")
                nc.sync.dma_start(t[:], vwT_d[d * 128:(d + 1) * 128, :])
                wv.append(t)
            lw_t = []
            for h in range(HPC):
                t = p_c.tile([64, NUM_OUT], F32R, name=f"lw{h}", tag=f"lw{h}")
                nc.sync.dma_start(t[:], lwT_d[h * 64:(h + 1) * 64, :])
                lw_t.append(t)

            # SBUF destinations for the projections
            qkT = {"q": [], "k": []}
            for nm in ("q", "k"):
                for pr in range(PAIRS):
                    qkT[nm].append(p_qk.tile([128, S], F32R,
                                             name=f"{nm}T{pr}", tag="qk"))
            v_t = [p_v.tile([128, HPC * P], F32R, name=f"v{st}", tag="v")
                   for st in range(ST)]

            # ---- projection work units (transient PSUM, usable as filler)
            def qk_unit(nm, pr, c):
                # qT/kT[p_hp, s] = sum_d wT[d, p_hp] * xt[d, s] for one
                # 512-wide s-chunk
                wts = wq if nm == "q" else wk
                bias = qb_t if nm == "q" else kb_t
                ps = p_pf.tile([128, 512], F32, name=f"pp_{nm}{pr}{c}",
                               tag="pf")
                for d in range(DT):
                    nc.tensor.matmul(
                        ps[:], wts[d][:, pr * 128:(pr + 1) * 128],
                        xt[d][:, c * 512:(c + 1) * 512],
                        start=(d == 0), stop=(d == DT - 1))
                nc.vector.tensor_scalar_add(
                    qkT[nm][pr][:, c * 512:(c + 1) * 512], ps[:],
                    bias[pr][:])

            def v_unit(st):
                # v[s, hp] = sum_d xt[d, s] * vwT[d, hp]  (+ ones^T @ vb)
                ps = p_pf.tile([128, 512], F32, name=f"pp_v{st}", tag="pf")
                for d in range(DT):
                    nc.tensor.matmul(
                        ps[:, :HPC * P],
                        xt[d][:, st * 128:(st + 1) * 128], wv[d][:],
                        start=(d == 0), stop=False)
                nc.tensor.matmul(ps[:, :HPC * P], ones[:], vb_t[:],
                                 start=False, stop=True)
                nc.vector.tensor_copy(v_t[st][:], ps[:, :HPC * P])

            # pair-0 q/k first: attention on head 0 starts as soon as the
            # input DMAs land
            for nm in ("q", "k"):
                for c in range(SC):
                    qk_unit(nm, 0, c)

            # remaining projections become filler inside heads 0/1:
            # head 0 runs the 16 v units (v_t[t] is ready just before the
            # ctx group that consumes it), head 1 runs pair-1 q/k.
            filler = {0: [lambda st=st: v_unit(st) for st in range(ST)],
                      1: [lambda nm=nm, c=c: qk_unit(nm, 1, c)
                          for nm in ("q", "k") for c in range(SC)]}

            # ---- attention, one head at a time, software-pipelined ----
            # scoresT[k_i, q_i] = sum_p kT[p, k_i] * qT[p, q_i]   (K=64)
            # eT = exp(scoresT/8) with fused Z = sum_q (scalar engine)
            # v' = v * (1/Z) per k row; ctx group: for each q-chunk c,
            # PSUM-accumulate 4 ki-tiles of vs^T-stationary @ eT, then
            # drain into the SBUF accumulator.
            ctxT = []
            for h in range(HPC):
                pr, off = divmod(h, 2)
                off *= 64
                kT, qT = qkT["k"][pr], qkT["q"][pr]
                fill = filler.get(h, [])
                acc = p_cc.tile([64, S], F32R, name=f"ctxT{h}", tag="cc")
                ets, zrs = [], []
                for t in range(ST):
                    pa = p_mm.tile([128, 1024], F32, name=f"ps_s{h}{t}a",
                                   tag="mm")
                    pb = p_mm.tile([128, 1024], F32, name=f"ps_s{h}{t}b",
                                   tag="mm")
                    lhsT = kT[off:off + 64, t * 128:(t + 1) * 128]
                    for c, (pt, o2) in enumerate(
                            ((pa, 0), (pa, 512), (pb, 0), (pb, 512))):
                        nc.tensor.matmul(
                            pt[:, o2:o2 + 512], lhsT,
                            qT[off:off + 64, c * 512:(c + 1) * 512],
                            start=True, stop=True)
                    zp = p_z.tile([128, 2], F32, name=f"zp{h}{t}", tag="zp")
                    et0 = p_et.tile([128, 1024], BF16, name=f"et{h}{t}a",
                                    tag="et")
                    et1 = p_et.tile([128, 1024], BF16, name=f"et{h}{t}b",
                                    tag="et")
                    nc.scalar.activation(et0[:], pa[:], EXP, scale=0.125,
                                         accum_out=zp[:, 0:1])
                    nc.scalar.activation(et1[:], pb[:], EXP, scale=0.125,
                                         accum_out=zp[:, 1:2])
                    z = p_z.tile([128, 1], F32, name=f"z{h}{t}", tag="z")
                    nc.vector.reduce_sum(z[:], zp[:], axis=AX)
                    zr = p_z.tile([128, 1], F32, name=f"zr{h}{t}", tag="zr")
                    nc.vector.reciprocal(zr[:], z[:])
                    ets.append((et0, et1))
                    zrs.append(zr)

                    if fill:
                        fill.pop(0)()

                    if t % GRP == GRP - 1:
                        g0 = t - (GRP - 1)
                        vss = []
                        for tt in range(g0, t + 1):
                            vs = p_z.tile([128, 64], BF16, name=f"vs{h}{tt}",
                                          tag="vs")
                            nc.vector.tensor_scalar_mul(
                                vs[:], v_t[tt][:, h * 64:(h + 1) * 64],
                                zrs[tt][:])
                            vss.append(vs)
                        for c in range(SC):
                            cp = p_cx.tile([64, 512], F32,
                                           name=f"cx{h}{t}{c}", tag="cx")
                            for i, tt in enumerate(range(g0, t + 1)):
                                nc.tensor.matmul(
                                    cp[:], vss[i],
                                    ets[tt][c // 2][:, (c % 2) * 512:
                                                    (c % 2) * 512 + 512],
                                    start=(i == 0), stop=(i == GRP - 1))
                            dsl = acc[:, c * 512:(c + 1) * 512]
                            if g0 == 0:
                                nc.vector.tensor_copy(dsl, cp[:])
                            else:
                                nc.vector.tensor_add(dsl, dsl, cp[:])
                while fill:
                    fill.pop(0)()
                ctxT.append(acc)

            # ---- output projection ----
            # out[s, n] = sum_h sum_p ctxT_h[p, s] * lwT_h[p, n]
            for st in range(ST):
                ob = p_ob.tile([128, NUM_OUT], F32, name=f"ob{st}", tag="ob")
                po = p_mm.tile([128, 1024], F32, name=f"ps_o{st}", tag="mm")
                for ncn in range(NC_CH):
                    o2 = ncn * 512
                    for h in range(HPC):
                        nc.tensor.matmul(
                            po[:, o2:o2 + 512],
                            ctxT[h][:, st * 128:(st + 1) * 128],
                            lw_t[h][:, ncn * 512:(ncn + 1) * 512],
                            start=(h == 0), stop=(h == HPC - 1))
                    nc.vector.tensor_copy(ob[:, o2:o2 + 512],
                                          po[:, o2:o2 + 512])
                nc.sync.dma_start(out_d[st * 128:(st + 1) * 128, :], ob[:])

    nc.compile()
    return nc


_NC_CACHE = None


def _get_nc():
    global _NC_CACHE
    if _NC_CACHE is None:
        _NC_CACHE = build_nc()
    return _NC_CACHE


def _prep_in_maps(x, q_w, q_b, k_w, k_b, v_w, v_b, l_w):
    """Host-side sharding: per-core input dict (core = b*4 + g)."""
    in_maps = []
    xts = [np.ascontiguousarray(x[b].T) for b in range(B)]
    ones = np.ones((1, 128), dtype=np.float32)
    for b in range(B):
        for g in range(4):
            hs = slice(g * HPC, (g + 1) * HPC)
            f0, f1 = g * HPC * P, (g + 1) * HPC * P
            in_maps.append({
                "xt": xts[b],
                "qwT": np.ascontiguousarray(
                    q_w[hs].transpose(2, 0, 1).reshape(D, HPC * P)),
                "kwT": np.ascontiguousarray(
                    k_w[hs].transpose(2, 0, 1).reshape(D, HPC * P)),
                "vwT": np.ascontiguousarray(
                    v_w[hs].transpose(2, 0, 1).reshape(D, HPC * P)),
                "qb": np.ascontiguousarray(q_b[hs].reshape(HPC * P, 1)),
                "kb": np.ascontiguousarray(k_b[hs].reshape(HPC * P, 1)),
                "vb": np.ascontiguousarray(v_b[hs].reshape(1, HPC * P)),
                "lwT": np.ascontiguousarray(l_w[:, f0:f1].T),
                "ones": ones,
            })
    return in_maps


def _run(inputs, trace=False):
    f32 = lambda a: np.asarray(a, dtype=np.float32)
    x = f32(inputs["x"])
    l_b = f32(inputs["l_b"])
    in_maps = _prep_in_maps(
        x, f32(inputs["q_w"]), f32(inputs["q_b"]), f32(inputs["k_w"]),
        f32(inputs["k_b"]), f32(inputs["v_w"]), f32(inputs["v_b"]),
        f32(inputs["l_w"]))
    nc = _get_nc()
    res = run_bass_kernel_spmd(nc, in_maps, list(range(N_CORES)), trace=trace)
    out = np.empty((B, S, NUM_OUT), dtype=np.float32)
    for b in range(B):
        acc = res.results[b * 4]["out"].astype(np.float32)
        for g in range(1, 4):
            acc = acc + res.results[b * 4 + g]["out"]
        out[b] = acc + l_b
    return out, res


def kernel(**inputs):
    out, _ = _run(inputs, trace=False)
    return out


# revision 13
# speedup vs baseline: 1.1376x; 1.1146x over previous
"""Trainium2 Bass kernel for MyMultiAttentionLayer.

Model (reference):
    q = einsum('bsd,hpd->bhsp', x, q_w) + q_b      (same for k, v)
    scores = q @ k^T / sqrt(P)                      [B,H,S,S]
    attn = softmax(scores, axis=2)                  # softmax over the QUERY axis
    ctx = einsum('bhqk,bhkp->bqhp', attn, v)
    out = concat(ctx) @ l_w.T + l_b                 [B,S,NUM_OUT]

Shapes: B=2, S=2048, D=1024, H=16, P=64, NUM_OUT=1024.

Sharding: 8 cores = 2 batches x 4 head-groups (4 heads each).  Each core
computes its batch's attention for its 4 heads plus the partial output
projection over its 256 features; the host sums the 4 partials per batch
(all-reduce equivalent) and adds l_b.

Softmax is over the query axis, so the normalizer Z[k] = sum_q exp(s[q,k])
depends only on k: ctx = sum_k e[q,k]*(v[k,:]/Z[k]) — the normalization is
folded into the 64-wide v rows instead of the 2048-wide attention matrix.
exp() runs on the scalar engine over [128,1024] PSUM blocks with a fused
free-axis accumulate producing Z; cost (N+352)/1.2 ns makes the scalar
engine the attention-phase throughput limit at ~2.9us per ki-tile.

The PE executes its stream in order and the HAM clock gate drops it to
1.2 GHz after ~3.4us of idle, so the schedule is built to keep the PE
continuously busy:
  * the pair-0 q/k projections run first (DMA-paced),
  * the v projection and pair-1 q/k projections are chopped into
    transient-PSUM units (8-9 matmuls each) and interleaved into the
    attention loop of heads 0/1 as filler the PE can chew while the
    scalar engine works off exp backlog,
  * ctx accumulates in PSUM in groups of 4 ki-tiles, drained to an fp32
    SBUF accumulator by the vector engine (frees 2 PSUM banks for the
    filler units),
  * attention scores for ki-tile t are emitted before the ctx matmuls of
    the previous group (software pipelining; the PE never waits on the
    exp of the tile it just produced).

Precision: projections / scores / output projection in float32r (fp32
data, fast PE mode); exp outputs and v/Z in bf16 (feed only the attn @ v
matmul; ~2e-3 relative error total).

Per-core layouts (transposes are done host-side when staging inputs):
  xt  [D,S]   = x[b].T                    (contraction dim d on partitions)
  qwT [D,4P]  (d, (h,p))                  kwT same, vwT same
  qb  [4P,1]  kb [4P,1]  vb [1,4P]
  lwT [4P,NUM_OUT] = l_w[:, feat_slice].T
  out [S,NUM_OUT] partial (no l_b)
"""

import numpy as np

import concourse.bass as bass
import concourse.tile as tile
from concourse import bacc, mybir
from concourse.bass_utils import run_bass_kernel_spmd

B, S, D = 2, 2048, 1024
H, P = 16, 64
NUM_OUT = 1024
N_CORES = 8
HPC = 4                 # heads per core
PAIRS = 2               # head pairs per core (2 heads x 64 = 128 partitions)
DT = D // 128           # 8 d-tiles
ST = S // 128           # 16 s-tiles
SC = S // 512           # 4 s-chunks of 512
NC_CH = NUM_OUT // 512  # 2 output chunks
GRP = 4                 # ki-tiles per ctx PSUM accumulation group

F32 = mybir.dt.float32
F32R = mybir.dt.float32r
BF16 = mybir.dt.bfloat16
EXP = mybir.ActivationFunctionType.Exp
AX = mybir.AxisListType.X


def build_nc():
    nc = bacc.Bacc("TRN2", target_bir_lowering=False, debug=False,
                   num_devices=N_CORES)

    xt_d = nc.dram_tensor("xt", [D, S], F32R, kind="ExternalInput")
    qwT_d = nc.dram_tensor("qwT", [D, HPC * P], F32R, kind="ExternalInput")
    kwT_d = nc.dram_tensor("kwT", [D, HPC * P], F32R, kind="ExternalInput")
    vwT_d = nc.dram_tensor("vwT", [D, HPC * P], F32R, kind="ExternalInput")
    qb_d = nc.dram_tensor("qb", [HPC * P, 1], F32, kind="ExternalInput")
    kb_d = nc.dram_tensor("kb", [HPC * P, 1], F32, kind="ExternalInput")
    vb_d = nc.dram_tensor("vb", [1, HPC * P], F32R, kind="ExternalInput")
    lwT_d = nc.dram_tensor("lwT", [HPC * P, NUM_OUT], F32R, kind="ExternalInput")
    ones_d = nc.dram_tensor("ones", [1, 128], F32R, kind="ExternalInput")
    out_d = nc.dram_tensor("out", [S, NUM_OUT], F32, kind="ExternalOutput")

    with tile.TileContext(nc) as tc:
        with (
            tc.tile_pool(name="qk", bufs=4) as p_qk,
            tc.tile_pool(name="vv", bufs=ST) as p_v,
            tc.tile_pool(name="cst", bufs=1) as p_c,
            tc.tile_pool(name="zz", bufs=6) as p_z,
            tc.tile_pool(name="et", bufs=10) as p_et,
            tc.tile_pool(name="cc", bufs=HPC) as p_cc,
            tc.tile_pool(name="ob", bufs=2) as p_ob,
            tc.tile_pool(name="xt", bufs=DT) as p_xt,
            tc.tile_pool(name="wst", bufs=3 * DT) as p_w,
            tc.tile_pool(name="mm", bufs=2, space=bass.MemorySpace.PSUM) as p_mm,
            tc.tile_pool(name="cx", bufs=2, space=bass.MemorySpace.PSUM) as p_cx,
            tc.tile_pool(name="pf", bufs=2, space=bass.MemorySpace.PSUM) as p_pf,
        ):
            # ---- stage inputs, in the order the PE needs them ----
            xt, wq, wk, wv = [], [], [], []
            for d in range(DT):
                t = p_w.tile([128, HPC * P], F32R, name=f"qw{d}", tag="w")
                nc.sync.dma_start(t[:], qwT_d[d * 128:(d + 1) * 128, :])
                wq.append(t)
                t = p_w.tile([128, HPC * P], F32R, name=f"kw{d}", tag="w")
                nc.sync.dma_start(t[:], kwT_d[d * 128:(d + 1) * 128, :])
                wk.append(t)
                t = p_xt.tile([128, S], F32R, name=f"xt{d}", tag="xt")
                nc.sync.dma_start(t[:], xt_d[d * 128:(d + 1) * 128, :])
                xt.append(t)
            qb_t, kb_t = [], []
            for pr in range(PAIRS):
                t = p_c.tile([128, 1], F32, name=f"qb{pr}", tag=f"qb{pr}")
                nc.sync.dma_start(t[:], qb_d[pr * 128:(pr + 1) * 128, :])
                qb_t.append(t)
                t = p_c.tile([128, 1], F32, name=f"kb{pr}", tag=f"kb{pr}")
                nc.sync.dma_start(t[:], kb_d[pr * 128:(pr + 1) * 128, :])
                kb_t.append(t)
            vb_t = p_c.tile([1, HPC * P], F32R, name="vb", tag="vb")
            nc.sync.dma_start(vb_t[:], vb_d[:, :])
            ones = p_c.tile([1, 128], F32R, name="ones", tag="ones")
            nc.sync.dma_start(ones[:], ones_d[:, :])
            for d in range(DT):
                t = p_w.tile([128, HPC * P], F32R, name=f"vw{d}", tag="w")
                nc.sync.dma_start(t[:], vwT_d[d * 128:(d + 1) * 128, :])
                wv.append(t)
            lw_t = []
            for h in range(HPC):
                t = p_c.tile([64, NUM_OUT], F32R, name=f"lw{h}", tag=f"lw{h}")
                nc.sync.dma_start(t[:], lwT_d[h * 64:(h + 1) * 64, :])
                lw_t.append(t)

            # SBUF destinations for the projections
            qkT = {"q": [], "k": []}
            for nm in ("q", "k"):
                for pr in range(PAIRS):
                    qkT[nm].append(p_qk.tile([128, S], F32R,
                                             name=f"{nm}T{pr}", tag="qk"))
            v_t = [p_v.tile([128, HPC * P], BF16, name=f"v{st}", tag="v")
                   for st in range(ST)]

            # ---- projection work units (transient PSUM, usable as filler)
            def qk_unit(nm, pr, c):
                # qT/kT[p_hp, s] = sum_d wT[d, p_hp] * xt[d, s] for one
                # 512-wide s-chunk
                wts = wq if nm == "q" else wk
                bias = qb_t if nm == "q" else kb_t
                ps = p_pf.tile([128, 512], F32, name=f"pp_{nm}{pr}{c}",
                               tag="pf")
                for d in range(DT):
                    nc.tensor.matmul(
                        ps[:], wts[d][:, pr * 128:(pr + 1) * 128],
                        xt[d][:, c * 512:(c + 1) * 512],
                        start=(d == 0), stop=(d == DT - 1))
                nc.vector.tensor_scalar_add(
                    qkT[nm][pr][:, c * 512:(c + 1) * 512], ps[:],
                    bias[pr][:])

            def v_unit(st):
                # v[s, hp] = sum_d xt[d, s] * vwT[d, hp]  (+ ones^T @ vb)
                ps = p_pf.tile([128, 512], F32, name=f"pp_v{st}", tag="pf")
                for d in range(DT):
                    nc.tensor.matmul(
                        ps[:, :HPC * P],
                        xt[d][:, st * 128:(st + 1) * 128], wv[d][:],
                        start=(d == 0), stop=False)
                nc.tensor.matmul(ps[:, :HPC * P], ones[:], vb_t[:],
                                 start=False, stop=True)
                nc.vector.tensor_copy(v_t[st][:], ps[:, :HPC * P])

            # pair-0 q/k first: attention on head 0 starts as soon as the
            # input DMAs land
            for nm in ("q", "k"):
                for c in range(SC):
                    qk_unit(nm, 0, c)

            # remaining projections become filler inside heads 0/1:
            # head 0 runs the 16 v units (v_t[t] is ready just before the
            # ctx group that consumes it), head 1 runs pair-1 q/k.
            filler = {0: [lambda st=st: v_unit(st) for st in range(ST)],
                      1: [lambda nm=nm, c=c: qk_unit(nm, 1, c)
                          for nm in ("q", "k") for c in range(SC)]}

            # ---- attention: uniform per-iteration emission ----
            # scoresT[k_i, q_i] = sum_p kT[p, k_i] * qT[p, q_i]   (K=64)
            # eT = exp(scoresT/8) with fused Z = sum_q (scalar engine)
            # v' = v * (1/Z) per k row.  ctx accumulates in PSUM over
            # groups of 4 ki-tiles; exactly ONE ctx q-chunk (4 matmuls +
            # a vector-engine drain into the SBUF accumulator) is emitted
            # per iteration, between the scores matmuls and the exps, so
            # the PE stream is uniform and et slots recycle steadily.
            ctxT = []
            chunk_queue = []

            def emit_chunk():
                h0_, g0, ets_g, vss_g, acc_, c = chunk_queue.pop(0)
                cp = p_cx.tile([64, 512], F32, name=f"cx{h0_}{g0}{c}",
                               tag="cx")
                for i in range(GRP):
                    nc.tensor.matmul(
                        cp[:], vss_g[i],
                        ets_g[i][c // 2][:, (c % 2) * 512:(c % 2) * 512 + 512],
                        start=(i == 0), stop=(i == GRP - 1))
                dsl = acc_[:, c * 512:(c + 1) * 512]
                if g0 == 0:
                    nc.vector.tensor_copy(dsl, cp[:])
                else:
                    nc.vector.tensor_add(dsl, dsl, cp[:])

            for h in range(HPC):
                pr, off = divmod(h, 2)
                off *= 64
                kT, qT = qkT["k"][pr], qkT["q"][pr]
                fill = filler.get(h, [])
                acc = p_cc.tile([64, S], F32R, name=f"ctxT{h}", tag="cc")
                ets, vss = [], []
                for t in range(ST):
                    pa = p_mm.tile([128, 1024], F32, name=f"ps_s{h}{t}a",
                                   tag="mm")
                    pb = p_mm.tile([128, 1024], F32, name=f"ps_s{h}{t}b",
                                   tag="mm")
                    lhsT = kT[off:off + 64, t * 128:(t + 1) * 128]
                    for c, (pt, o2) in enumerate(
                            ((pa, 0), (pa, 512), (pb, 0), (pb, 512))):
                        nc.tensor.matmul(
                            pt[:, o2:o2 + 512], lhsT,
                            qT[off:off + 64, c * 512:(c + 1) * 512],
                            start=True, stop=True)
                    if chunk_queue:
                        emit_chunk()
                    zp = p_z.tile([128, 2], F32, name=f"zp{h}{t}", tag="zp",
                                  bufs=4)
                    et0 = p_et.tile([128, 1024], BF16, name=f"et{h}{t}a",
                                    tag="et")
                    et1 = p_et.tile([128, 1024], BF16, name=f"et{h}{t}b",
                                    tag="et")
                    nc.scalar.activation(et0[:], pa[:], EXP, scale=0.125,
                                         accum_out=zp[:, 0:1])
                    nc.scalar.activation(et1[:], pb[:], EXP, scale=0.125,
                                         accum_out=zp[:, 1:2])
                    if fill:
                        fill.pop(0)()
                    z = p_z.tile([128, 1], F32, name=f"z{h}{t}", tag="z",
                                 bufs=4)
                    nc.vector.reduce_sum(z[:], zp[:], axis=AX)
                    zr = p_z.tile([128, 1], F32, name=f"zr{h}{t}", tag="zr",
                                  bufs=4)
                    nc.vector.reciprocal(zr[:], z[:])
                    vs = p_z.tile([128, 64], BF16, name=f"vs{h}{t}",
                                  tag="vs", bufs=12)
                    nc.vector.tensor_scalar_mul(
                        vs[:], v_t[t][:, h * 64:(h + 1) * 64], zr[:])
                    ets.append((et0, et1))
                    vss.append(vs)
                    if t % GRP == GRP - 1:
                        g0 = t - (GRP - 1)
                        for c in range(SC):
                            chunk_queue.append(
                                (h, g0, ets[g0:t + 1], vss[g0:t + 1], acc, c))
                while fill:
                    fill.pop(0)()
                ctxT.append(acc)
            while chunk_queue:
                emit_chunk()

            # ---- output projection ----
            # out[s, n] = sum_h sum_p ctxT_h[p, s] * lwT_h[p, n]
            for st in range(ST):
                ob = p_ob.tile([128, NUM_OUT], F32, name=f"ob{st}", tag="ob")
                po = p_mm.tile([128, 1024], F32, name=f"ps_o{st}", tag="mm")
                for ncn in range(NC_CH):
                    o2 = ncn * 512
                    for h in range(HPC):
                        nc.tensor.matmul(
                            po[:, o2:o2 + 512],
                            ctxT[h][:, st * 128:(st + 1) * 128],
                            lw_t[h][:, ncn * 512:(ncn + 1) * 512],
                            start=(h == 0), stop=(h == HPC - 1))
                    nc.vector.tensor_copy(ob[:, o2:o2 + 512],
                                          po[:, o2:o2 + 512])
                nc.sync.dma_start(out_d[st * 128:(st + 1) * 128, :], ob[:])

    nc.compile()
    return nc


_NC_CACHE = None


def _get_nc():
    global _NC_CACHE
    if _NC_CACHE is None:
        _NC_CACHE = build_nc()
    return _NC_CACHE


def _prep_in_maps(x, q_w, q_b, k_w, k_b, v_w, v_b, l_w):
    """Host-side sharding: per-core input dict (core = b*4 + g)."""
    in_maps = []
    xts = [np.ascontiguousarray(x[b].T) for b in range(B)]
    ones = np.ones((1, 128), dtype=np.float32)
    for b in range(B):
        for g in range(4):
            hs = slice(g * HPC, (g + 1) * HPC)
            f0, f1 = g * HPC * P, (g + 1) * HPC * P
            in_maps.append({
                "xt": xts[b],
                "qwT": np.ascontiguousarray(
                    q_w[hs].transpose(2, 0, 1).reshape(D, HPC * P)),
                "kwT": np.ascontiguousarray(
                    k_w[hs].transpose(2, 0, 1).reshape(D, HPC * P)),
                "vwT": np.ascontiguousarray(
                    v_w[hs].transpose(2, 0, 1).reshape(D, HPC * P)),
                "qb": np.ascontiguousarray(q_b[hs].reshape(HPC * P, 1)),
                "kb": np.ascontiguousarray(k_b[hs].reshape(HPC * P, 1)),
                "vb": np.ascontiguousarray(v_b[hs].reshape(1, HPC * P)),
                "lwT": np.ascontiguousarray(l_w[:, f0:f1].T),
                "ones": ones,
            })
    return in_maps


def _run(inputs, trace=False):
    f32 = lambda a: np.asarray(a, dtype=np.float32)
    x = f32(inputs["x"])
    l_b = f32(inputs["l_b"])
    in_maps = _prep_in_maps(
        x, f32(inputs["q_w"]), f32(inputs["q_b"]), f32(inputs["k_w"]),
        f32(inputs["k_b"]), f32(inputs["v_w"]), f32(inputs["v_b"]),
        f32(inputs["l_w"]))
    nc = _get_nc()
    res = run_bass_kernel_spmd(nc, in_maps, list(range(N_CORES)), trace=trace)
    out = np.empty((B, S, NUM_OUT), dtype=np.float32)
    for b in range(B):
        acc = res.results[b * 4]["out"].astype(np.float32)
        for g in range(1, 4):
            acc = acc + res.results[b * 4 + g]["out"]
        out[b] = acc + l_b
    return out, res


def kernel(**inputs):
    out, _ = _run(inputs, trace=False)
    return out


# revision 14
# speedup vs baseline: 1.1892x; 1.0453x over previous
"""Trainium2 Bass kernel for MyMultiAttentionLayer.

Model (reference):
    q = einsum('bsd,hpd->bhsp', x, q_w) + q_b      (same for k, v)
    scores = q @ k^T / sqrt(P)                      [B,H,S,S]
    attn = softmax(scores, axis=2)                  # softmax over the QUERY axis
    ctx = einsum('bhqk,bhkp->bqhp', attn, v)
    out = concat(ctx) @ l_w.T + l_b                 [B,S,NUM_OUT]

Shapes: B=2, S=2048, D=1024, H=16, P=64, NUM_OUT=1024.

Sharding: 8 cores = 2 batches x 4 head-groups (4 heads each).  Each core
computes its batch's attention for its 4 heads plus the partial output
projection over its 256 features; the host sums the 4 partials per batch
(all-reduce equivalent) and adds l_b.

Softmax is over the query axis, so the normalizer Z[k] = sum_q exp(s[q,k])
depends only on k: ctx = sum_k e[q,k]*(v[k,:]/Z[k]) — the normalization is
folded into the 64-wide v rows instead of the 2048-wide attention matrix.

Precision: all matmul inputs are fp16 (11-bit mantissa — same relative
precision as the PE's fast-fp32 "float32r" mode, but with pipelined
weight loads instead of a serial per-matmul reload) accumulated in fp32
PSUM; exp outputs in bf16 (exp range overflows fp16).

Schedule (the PE executes its stream in order, and the HAM clock gate
halves the PE clock unless the PE stays continuously busy):
  * pair-0 q/k projections run first (DMA-paced),
  * scores for one ki-tile are 4 matmuls into one [128,2048] PSUM tile;
    ONE wide exp ACTIVATE (with fused Z accumulate) consumes it —
    (N+352)/1.2ns makes this the cheapest exp schedule at ~2.3us/tile,
  * ctx accumulates in PSUM over groups of 4 ki-tiles; exactly one ctx
    q-chunk (4 matmuls + vector-engine drain into an SBUF accumulator)
    is emitted per iteration, between scores and exp, so et buffers
    recycle steadily and the PE stream is uniform,
  * v projection and pair-1 q/k projections are chopped into transient-
    PSUM units and interleaved as PE filler in heads 0/1; iterations with
    no real filler emit two dummy matmuls to keep the PE saturated (an
    idle PE gets clock-gated to 1.2 GHz, doubling every matmul).

Per-core layouts (transposes are done host-side when staging inputs):
  xt  [D,S]   = x[b].T  fp16              (contraction dim d on partitions)
  qwT [D,4P]  (d, (h,p)) fp16             kwT same, vwT same
  qb  [4P,1]  kb [4P,1]  fp32, vb [1,4P] fp16
  lwT [4P,NUM_OUT] = l_w[:, feat_slice].T fp16
  out [S,NUM_OUT] fp32 partial (no l_b)
"""

import numpy as np

import concourse.bass as bass
import concourse.tile as tile
from concourse import bacc, mybir
from concourse.bass_utils import run_bass_kernel_spmd

B, S, D = 2, 2048, 1024
H, P = 16, 64
NUM_OUT = 1024
N_CORES = 8
HPC = 4                 # heads per core
PAIRS = 2               # head pairs per core (2 heads x 64 = 128 partitions)
DT = D // 128           # 8 d-tiles
ST = S // 128           # 16 s-tiles
SC = S // 512           # 4 s-chunks of 512
NC_CH = NUM_OUT // 512  # 2 output chunks
GRP = 4                 # ki-tiles per ctx PSUM accumulation group

F32 = mybir.dt.float32
F16 = mybir.dt.float16
BF16 = mybir.dt.bfloat16
EXP = mybir.ActivationFunctionType.Exp


def build_nc():
    nc = bacc.Bacc("TRN2", target_bir_lowering=False, debug=False,
                   num_devices=N_CORES)

    xt_d = nc.dram_tensor("xt", [D, S], F16, kind="ExternalInput")
    qwT_d = nc.dram_tensor("qwT", [D, HPC * P], F16, kind="ExternalInput")
    kwT_d = nc.dram_tensor("kwT", [D, HPC * P], F16, kind="ExternalInput")
    vwT_d = nc.dram_tensor("vwT", [D, HPC * P], F16, kind="ExternalInput")
    qb_d = nc.dram_tensor("qb", [HPC * P, 1], F32, kind="ExternalInput")
    kb_d = nc.dram_tensor("kb", [HPC * P, 1], F32, kind="ExternalInput")
    vb_d = nc.dram_tensor("vb", [1, HPC * P], F16, kind="ExternalInput")
    lwT_d = nc.dram_tensor("lwT", [HPC * P, NUM_OUT], F16, kind="ExternalInput")
    ones_d = nc.dram_tensor("ones", [1, 128], F16, kind="ExternalInput")
    out_d = nc.dram_tensor("out", [S, NUM_OUT], F32, kind="ExternalOutput")

    with tile.TileContext(nc) as tc:
        with (
            tc.tile_pool(name="qk", bufs=4) as p_qk,
            tc.tile_pool(name="vv", bufs=ST) as p_v,
            tc.tile_pool(name="cst", bufs=1) as p_c,
            tc.tile_pool(name="zz", bufs=6) as p_z,
            tc.tile_pool(name="et", bufs=8) as p_et,
            tc.tile_pool(name="cc", bufs=HPC) as p_cc,
            tc.tile_pool(name="ob", bufs=2) as p_ob,
            tc.tile_pool(name="xt", bufs=DT) as p_xt,
            tc.tile_pool(name="wst", bufs=3 * DT) as p_w,
            tc.tile_pool(name="mm", bufs=1, space=bass.MemorySpace.PSUM) as p_mm,
            tc.tile_pool(name="cx", bufs=2, space=bass.MemorySpace.PSUM) as p_cx,
            tc.tile_pool(name="pf", bufs=2, space=bass.MemorySpace.PSUM) as p_pf,
        ):
            # ---- stage inputs, in the order the PE needs them ----
            xt, wq, wk, wv = [], [], [], []
            for d in range(DT):
                t = p_w.tile([128, HPC * P], F16, name=f"qw{d}", tag="w")
                nc.sync.dma_start(t[:], qwT_d[d * 128:(d + 1) * 128, :])
                wq.append(t)
                t = p_w.tile([128, HPC * P], F16, name=f"kw{d}", tag="w")
                nc.sync.dma_start(t[:], kwT_d[d * 128:(d + 1) * 128, :])
                wk.append(t)
                t = p_xt.tile([128, S], F16, name=f"xt{d}", tag="xt")
                nc.sync.dma_start(t[:], xt_d[d * 128:(d + 1) * 128, :])
                xt.append(t)
            qb_t, kb_t = [], []
            for pr in range(PAIRS):
                t = p_c.tile([128, 1], F32, name=f"qb{pr}", tag=f"qb{pr}")
                nc.sync.dma_start(t[:], qb_d[pr * 128:(pr + 1) * 128, :])
                qb_t.append(t)
                t = p_c.tile([128, 1], F32, name=f"kb{pr}", tag=f"kb{pr}")
                nc.sync.dma_start(t[:], kb_d[pr * 128:(pr + 1) * 128, :])
                kb_t.append(t)
            vb_t = p_c.tile([1, HPC * P], F16, name="vb", tag="vb")
            nc.sync.dma_start(vb_t[:], vb_d[:, :])
            ones = p_c.tile([1, 128], F16, name="ones", tag="ones")
            nc.sync.dma_start(ones[:], ones_d[:, :])
            for d in range(DT):
                t = p_w.tile([128, HPC * P], F16, name=f"vw{d}", tag="w")
                nc.sync.dma_start(t[:], vwT_d[d * 128:(d + 1) * 128, :])
                wv.append(t)
            lw_t = []
            for h in range(HPC):
                t = p_c.tile([64, NUM_OUT], F16, name=f"lw{h}", tag=f"lw{h}")
                nc.sync.dma_start(t[:], lwT_d[h * 64:(h + 1) * 64, :])
                lw_t.append(t)

            # SBUF destinations for the projections
            qkT = {"q": [], "k": []}
            for nm in ("q", "k"):
                for pr in range(PAIRS):
                    qkT[nm].append(p_qk.tile([128, S], F16,
                                             name=f"{nm}T{pr}", tag="qk"))
            v_t = [p_v.tile([128, HPC * P], F16, name=f"v{st}", tag="v")
                   for st in range(ST)]

            # ---- projection work units (transient PSUM, usable as filler)
            def qk_unit(nm, pr, c):
                # qT/kT[p_hp, s] = sum_d wT[d, p_hp] * xt[d, s] for one
                # 512-wide s-chunk
                wts = wq if nm == "q" else wk
                bias = qb_t if nm == "q" else kb_t
                ps = p_pf.tile([128, 512], F32, name=f"pp_{nm}{pr}{c}",
                               tag="pf")
                for d in range(DT):
                    nc.tensor.matmul(
                        ps[:], wts[d][:, pr * 128:(pr + 1) * 128],
                        xt[d][:, c * 512:(c + 1) * 512],
                        start=(d == 0), stop=(d == DT - 1))
                nc.vector.tensor_scalar_add(
                    qkT[nm][pr][:, c * 512:(c + 1) * 512], ps[:],
                    bias[pr][:])

            def v_unit(st):
                # v[s, hp] = sum_d xt[d, s] * vwT[d, hp]  (+ ones^T @ vb)
                ps = p_pf.tile([128, 512], F32, name=f"pp_v{st}", tag="pf")
                for d in range(DT):
                    nc.tensor.matmul(
                        ps[:, :HPC * P],
                        xt[d][:, st * 128:(st + 1) * 128], wv[d][:],
                        start=(d == 0), stop=False)
                nc.tensor.matmul(ps[:, :HPC * P], ones[:], vb_t[:],
                                 start=False, stop=True)
                nc.vector.tensor_copy(v_t[st][:], ps[:, :HPC * P])

            ndum = [0]

            def dummy_unit(n=2):
                # keep-warm matmuls: the HAM activity monitor halves the PE
                # clock whenever the PE idles; burning ~0.5us here is far
                # cheaper than running every later matmul at 1.2 GHz.
                ps = p_pf.tile([128, 512], F32, name=f"pp_d{ndum[0]}",
                               tag="pf")
                ndum[0] += 1
                for i in range(n):
                    nc.tensor.matmul(ps[:], xt[0][:, :128], xt[0][:, :512],
                                     start=(i == 0), stop=(i == n - 1))

            # pair-0 q/k first: attention on head 0 starts as soon as the
            # input DMAs land
            for nm in ("q", "k"):
                for c in range(SC):
                    qk_unit(nm, 0, c)

            filler = {0: [lambda st=st: v_unit(st) for st in range(ST)],
                      1: [lambda nm=nm, c=c: qk_unit(nm, 1, c)
                          for nm in ("q", "k") for c in range(SC)]}

            # ---- attention: uniform per-iteration emission ----
            # scoresT[k_i, q_i] = sum_p kT[p, k_i] * qT[p, q_i]   (K=64)
            # one wide exp per ki-tile: eT = exp(scoresT/8), Z fused
            ctxT = []
            chunk_queue = []

            def emit_chunk():
                h0_, g0, ets_g, vss_g, acc_, c = chunk_queue.pop(0)
                cp = p_cx.tile([64, 512], F32, name=f"cx{h0_}{g0}{c}",
                               tag="cx")
                for i in range(GRP):
                    nc.tensor.matmul(
                        cp[:], vss_g[i],
                        ets_g[i][:, c * 512:(c + 1) * 512],
                        start=(i == 0), stop=(i == GRP - 1))
                dsl = acc_[:, c * 512:(c + 1) * 512]
                if g0 == 0:
                    nc.vector.tensor_copy(dsl, cp[:])
                else:
                    nc.vector.tensor_add(dsl, dsl, cp[:])

            for h in range(HPC):
                pr, off = divmod(h, 2)
                off *= 64
                kT, qT = qkT["k"][pr], qkT["q"][pr]
                fill = filler.get(h, [])
                acc = p_cc.tile([64, S], F16, name=f"ctxT{h}", tag="cc")
                ets, vss = [], []
                for t in range(ST):
                    pa = p_mm.tile([128, 2048], F32, name=f"ps_s{h}{t}",
                                   tag="mm")
                    lhsT = kT[off:off + 64, t * 128:(t + 1) * 128]
                    for c in range(SC):
                        nc.tensor.matmul(
                            pa[:, c * 512:(c + 1) * 512], lhsT,
                            qT[off:off + 64, c * 512:(c + 1) * 512],
                            start=True, stop=True)
                    if chunk_queue:
                        emit_chunk()
                    zp = p_z.tile([128, 1], F32, name=f"zp{h}{t}", tag="zp",
                                  bufs=4)
                    et0 = p_et.tile([128, 2048], BF16, name=f"et{h}{t}",
                                    tag="et")
                    nc.scalar.activation(et0[:], pa[:], EXP, scale=0.125,
                                         accum_out=zp[:])
                    if fill:
                        fill.pop(0)()
                    else:
                        dummy_unit()
                    zr = p_z.tile([128, 1], F32, name=f"zr{h}{t}", tag="zr",
                                  bufs=4)
                    nc.vector.reciprocal(zr[:], zp[:])
                    vs = p_z.tile([128, 64], BF16, name=f"vs{h}{t}",
                                  tag="vs", bufs=12)
                    nc.vector.tensor_scalar_mul(
                        vs[:], v_t[t][:, h * 64:(h + 1) * 64], zr[:])
                    ets.append(et0)
                    vss.append(vs)
                    if t % GRP == GRP - 1:
                        g0 = t - (GRP - 1)
                        for c in range(SC):
                            chunk_queue.append(
                                (h, g0, ets[g0:t + 1], vss[g0:t + 1], acc, c))
                while fill:
                    fill.pop(0)()
                ctxT.append(acc)
            while chunk_queue:
                emit_chunk()

            # ---- output projection ----
            # out[s, n] = sum_h sum_p ctxT_h[p, s] * lwT_h[p, n]
            for st in range(ST):
                ob = p_ob.tile([128, NUM_OUT], F32, name=f"ob{st}", tag="ob")
                for ncn in range(NC_CH):
                    pool, tg = (p_pf, "pf") if ncn == 0 else (p_cx, "cx")
                    po = pool.tile([128, 512], F32, name=f"ps_o{st}{ncn}",
                                   tag=tg)
                    for hh in range(HPC):
                        nc.tensor.matmul(
                            po[:],
                            ctxT[hh][:, st * 128:(st + 1) * 128],
                            lw_t[hh][:, ncn * 512:(ncn + 1) * 512],
                            start=(hh == 0), stop=(hh == HPC - 1))
                    nc.vector.tensor_copy(
                        ob[:, ncn * 512:(ncn + 1) * 512], po[:])
                nc.sync.dma_start(out_d[st * 128:(st + 1) * 128, :], ob[:])

    nc.compile()
    return nc


_NC_CACHE = None


def _get_nc():
    global _NC_CACHE
    if _NC_CACHE is None:
        _NC_CACHE = build_nc()
    return _NC_CACHE


def _prep_in_maps(x, q_w, q_b, k_w, k_b, v_w, v_b, l_w):
    """Host-side sharding: per-core input dict (core = b*4 + g)."""
    f16 = np.float16
    in_maps = []
    xts = [np.ascontiguousarray(x[b].T.astype(f16)) for b in range(B)]
    ones = np.ones((1, 128), dtype=f16)
    for b in range(B):
        for g in range(4):
            hs = slice(g * HPC, (g + 1) * HPC)
            f0, f1 = g * HPC * P, (g + 1) * HPC * P
            in_maps.append({
                "xt": xts[b],
                "qwT": np.ascontiguousarray(
                    q_w[hs].transpose(2, 0, 1).reshape(D, HPC * P)
                    .astype(f16)),
                "kwT": np.ascontiguousarray(
                    k_w[hs].transpose(2, 0, 1).reshape(D, HPC * P)
                    .astype(f16)),
                "vwT": np.ascontiguousarray(
                    v_w[hs].transpose(2, 0, 1).reshape(D, HPC * P)
                    .astype(f16)),
                "qb": np.ascontiguousarray(q_b[hs].reshape(HPC * P, 1)),
                "kb": np.ascontiguousarray(k_b[hs].reshape(HPC * P, 1)),
                "vb": np.ascontiguousarray(v_b[hs].reshape(1, HPC * P)
                                           .astype(f16)),
                "lwT": np.ascontiguousarray(l_w[:, f0:f1].T.astype(f16)),
                "ones": ones,
            })
    return in_maps


def _run(inputs, trace=False):
    f32 = lambda a: np.asarray(a, dtype=np.float32)
    x = f32(inputs["x"])
    l_b = f32(inputs["l_b"])
    in_maps = _prep_in_maps(
        x, f32(inputs["q_w"]), f32(inputs["q_b"]), f32(inputs["k_w"]),
        f32(inputs["k_b"]), f32(inputs["v_w"]), f32(inputs["v_b"]),
        f32(inputs["l_w"]))
    nc = _get_nc()
    res = run_bass_kernel_spmd(nc, in_maps, list(range(N_CORES)), trace=trace)
    out = np.empty((B, S, NUM_OUT), dtype=np.float32)
    for b in range(B):
        acc = res.results[b * 4]["out"].astype(np.float32)
        for g in range(1, 4):
            acc = acc + res.results[b * 4 + g]["out"]
        out[b] = acc + l_b
    return out, res


def kernel(**inputs):
    out, _ = _run(inputs, trace=False)
    return out


# revision 15
# speedup vs baseline: 1.4150x; 1.1899x over previous
"""Trainium2 Bass kernel for MyMultiAttentionLayer.

Model (reference):
    q = einsum('bsd,hpd->bhsp', x, q_w) + q_b      (same for k, v)
    scores = q @ k^T / sqrt(P)                      [B,H,S,S]
    attn = softmax(scores, axis=2)                  # softmax over the QUERY axis
    ctx = einsum('bhqk,bhkp->bqhp', attn, v)
    out = concat(ctx) @ l_w.T + l_b                 [B,S,NUM_OUT]

Shapes: B=2, S=2048, D=1024, H=16, P=64, NUM_OUT=1024.

Sharding: 8 cores = 2 batches x 4 head-groups (4 heads each).  Each core
computes its batch's attention for its 4 heads plus the partial output
projection over its 256 features; the host sums the 4 partials per batch
(all-reduce equivalent) and adds l_b.

Softmax is over the query axis, so the normalizer Z[k] = sum_q exp(s[q,k])
depends only on k: ctx = sum_k e[q,k]*(v[k,:]/Z[k]) — the normalization is
folded into the 64-wide v rows instead of the 2048-wide attention matrix.

Precision: all matmul inputs are fp16 (11-bit mantissa — same relative
precision as the PE's fast-fp32 "float32r" mode, but with pipelined
weight loads instead of a serial per-matmul reload) accumulated in fp32
PSUM; exp outputs in bf16 (exp range overflows fp16).

Schedule (the PE executes its stream in order, and the HAM clock gate
halves the PE clock unless the PE stays continuously busy):
  * pair-0 q/k projections run first (DMA-paced),
  * scores for one ki-tile are 4 matmuls into one [128,2048] PSUM tile;
    ONE wide exp ACTIVATE (with fused Z accumulate) consumes it —
    (N+352)/1.2ns makes this the cheapest exp schedule at ~2.3us/tile,
  * ctx accumulates in PSUM over groups of 4 ki-tiles; exactly one ctx
    q-chunk (4 matmuls + vector-engine drain into an SBUF accumulator)
    is emitted per iteration, between scores and exp, so et buffers
    recycle steadily and the PE stream is uniform,
  * v projection and pair-1 q/k projections are chopped into transient-
    PSUM units and interleaved as PE filler in heads 0/1; iterations with
    no real filler emit two dummy matmuls to keep the PE saturated (an
    idle PE gets clock-gated to 1.2 GHz, doubling every matmul).

Per-core layouts (transposes are done host-side when staging inputs):
  xt  [D,S]   = x[b].T  fp16              (contraction dim d on partitions)
  qwT [D,4P]  (d, (h,p)) fp16             kwT same, vwT same
  qb  [4P,1]  kb [4P,1]  fp32, vb [1,4P] fp16
  lwT [4P,NUM_OUT] = l_w[:, feat_slice].T fp16
  out [S,NUM_OUT] fp32 partial (no l_b)
"""

import numpy as np

import concourse.bass as bass
import concourse.tile as tile
from concourse import bacc, mybir
from concourse.bass_utils import run_bass_kernel_spmd

B, S, D = 2, 2048, 1024
H, P = 16, 64
NUM_OUT = 1024
N_CORES = 8
HPC = 4                 # heads per core
PAIRS = 2               # head pairs per core (2 heads x 64 = 128 partitions)
DT = D // 128           # 8 d-tiles
ST = S // 128           # 16 s-tiles
SC = S // 512           # 4 s-chunks of 512
NC_CH = NUM_OUT // 512  # 2 output chunks
GRP = 4                 # ki-tiles per ctx PSUM accumulation group

F32 = mybir.dt.float32
F16 = mybir.dt.float16
BF16 = mybir.dt.bfloat16
EXP = mybir.ActivationFunctionType.Exp


def build_nc():
    nc = bacc.Bacc("TRN2", target_bir_lowering=False, debug=False,
                   num_devices=N_CORES)

    xt_d = nc.dram_tensor("xt", [D, S], F16, kind="ExternalInput")
    qwT_d = nc.dram_tensor("qwT", [D, HPC * P], F16, kind="ExternalInput")
    kwT_d = nc.dram_tensor("kwT", [D, HPC * P], F16, kind="ExternalInput")
    vwT_d = nc.dram_tensor("vwT", [D, HPC * P], F16, kind="ExternalInput")
    qb_d = nc.dram_tensor("qb", [HPC * P, 1], F32, kind="ExternalInput")
    kb_d = nc.dram_tensor("kb", [HPC * P, 1], F32, kind="ExternalInput")
    vb_d = nc.dram_tensor("vb", [1, HPC * P], F16, kind="ExternalInput")
    lwT_d = nc.dram_tensor("lwT", [HPC * P, NUM_OUT], F16, kind="ExternalInput")
    ones_d = nc.dram_tensor("ones", [1, 128], F16, kind="ExternalInput")
    out_d = nc.dram_tensor("out", [S, NUM_OUT], F32, kind="ExternalOutput")

    with tile.TileContext(nc) as tc:
        with (
            tc.tile_pool(name="qk", bufs=4) as p_qk,
            tc.tile_pool(name="vv", bufs=ST) as p_v,
            tc.tile_pool(name="cst", bufs=1) as p_c,
            tc.tile_pool(name="zz", bufs=6) as p_z,
            tc.tile_pool(name="et", bufs=10) as p_et,
            tc.tile_pool(name="cc", bufs=HPC) as p_cc,
            tc.tile_pool(name="ob", bufs=2) as p_ob,
            tc.tile_pool(name="xt", bufs=DT) as p_xt,
            tc.tile_pool(name="wst", bufs=3 * DT) as p_w,
            tc.tile_pool(name="mm", bufs=2, space=bass.MemorySpace.PSUM) as p_mm,
            tc.tile_pool(name="cx", bufs=2, space=bass.MemorySpace.PSUM) as p_cx,
            tc.tile_pool(name="pf", bufs=2, space=bass.MemorySpace.PSUM) as p_pf,
        ):
            # ---- stage inputs, in the order the PE needs them ----
            xt, wq, wk, wv = [], [], [], []
            for d in range(DT):
                t = p_w.tile([128, HPC * P], F16, name=f"qw{d}", tag="w")
                nc.sync.dma_start(t[:], qwT_d[d * 128:(d + 1) * 128, :])
                wq.append(t)
                t = p_w.tile([128, HPC * P], F16, name=f"kw{d}", tag="w")
                nc.sync.dma_start(t[:], kwT_d[d * 128:(d + 1) * 128, :])
                wk.append(t)
                t = p_xt.tile([128, S], F16, name=f"xt{d}", tag="xt")
                nc.sync.dma_start(t[:], xt_d[d * 128:(d + 1) * 128, :])
                xt.append(t)
            qb_t, kb_t = [], []
            for pr in range(PAIRS):
                t = p_c.tile([128, 1], F32, name=f"qb{pr}", tag=f"qb{pr}")
                nc.sync.dma_start(t[:], qb_d[pr * 128:(pr + 1) * 128, :])
                qb_t.append(t)
                t = p_c.tile([128, 1], F32, name=f"kb{pr}", tag=f"kb{pr}")
                nc.sync.dma_start(t[:], kb_d[pr * 128:(pr + 1) * 128, :])
                kb_t.append(t)
            vb_t = p_c.tile([1, HPC * P], F16, name="vb", tag="vb")
            nc.sync.dma_start(vb_t[:], vb_d[:, :])
            ones = p_c.tile([1, 128], F16, name="ones", tag="ones")
            nc.sync.dma_start(ones[:], ones_d[:, :])
            for d in range(DT):
                t = p_w.tile([128, HPC * P], F16, name=f"vw{d}", tag="w")
                nc.sync.dma_start(t[:], vwT_d[d * 128:(d + 1) * 128, :])
                wv.append(t)
            lw_t = []
            for h in range(HPC):
                t = p_c.tile([64, NUM_OUT], F16, name=f"lw{h}", tag=f"lw{h}")
                nc.sync.dma_start(t[:], lwT_d[h * 64:(h + 1) * 64, :])
                lw_t.append(t)

            # SBUF destinations for the projections
            qkT = {"q": [], "k": []}
            for nm in ("q", "k"):
                for pr in range(PAIRS):
                    qkT[nm].append(p_qk.tile([128, S], F16,
                                             name=f"{nm}T{pr}", tag="qk"))
            v_t = [p_v.tile([128, HPC * P], F16, name=f"v{st}", tag="v")
                   for st in range(ST)]

            # ---- projection work units (transient PSUM, usable as filler)
            def qk_unit(nm, pr, c):
                # qT/kT[p_hp, s] = sum_d wT[d, p_hp] * xt[d, s] for one
                # 512-wide s-chunk
                wts = wq if nm == "q" else wk
                bias = qb_t if nm == "q" else kb_t
                ps = p_pf.tile([128, 512], F32, name=f"pp_{nm}{pr}{c}",
                               tag="pf")
                for d in range(DT):
                    nc.tensor.matmul(
                        ps[:], wts[d][:, pr * 128:(pr + 1) * 128],
                        xt[d][:, c * 512:(c + 1) * 512],
                        start=(d == 0), stop=(d == DT - 1))
                nc.vector.tensor_scalar_add(
                    qkT[nm][pr][:, c * 512:(c + 1) * 512], ps[:],
                    bias[pr][:])

            def v_unit(st):
                # v[s, hp] = sum_d xt[d, s] * vwT[d, hp]  (+ ones^T @ vb)
                ps = p_pf.tile([128, 512], F32, name=f"pp_v{st}", tag="pf")
                for d in range(DT):
                    nc.tensor.matmul(
                        ps[:, :HPC * P],
                        xt[d][:, st * 128:(st + 1) * 128], wv[d][:],
                        start=(d == 0), stop=False)
                nc.tensor.matmul(ps[:, :HPC * P], ones[:], vb_t[:],
                                 start=False, stop=True)
                nc.vector.tensor_copy(v_t[st][:], ps[:, :HPC * P])

            ndum = [0]

            def dummy_unit(n=2):
                # keep-warm matmuls: the HAM activity monitor halves the PE
                # clock whenever the PE idles; burning ~0.5us here is far
                # cheaper than running every later matmul at 1.2 GHz.
                ps = p_pf.tile([128, 512], F32, name=f"pp_d{ndum[0]}",
                               tag="pf")
                ndum[0] += 1
                for i in range(n):
                    nc.tensor.matmul(ps[:], xt[0][:, :128], xt[0][:, :512],
                                     start=(i == 0), stop=(i == n - 1))

            # pair-0 q/k first: attention on head 0 starts as soon as the
            # input DMAs land
            for nm in ("q", "k"):
                for c in range(SC):
                    qk_unit(nm, 0, c)

            filler = {0: [lambda st=st: v_unit(st) for st in range(ST)],
                      1: [lambda nm=nm, c=c: qk_unit(nm, 1, c)
                          for nm in ("q", "k") for c in range(SC)]}

            # ---- attention: uniform per-iteration emission ----
            # scoresT[k_i, q_i] = sum_p kT[p, k_i] * qT[p, q_i]   (K=64)
            # one wide exp per ki-tile: eT = exp(scoresT/8), Z fused
            ctxT = []
            chunk_queue = []

            def emit_chunk():
                h0_, g0, ets_g, vss_g, acc_, c = chunk_queue.pop(0)
                cp = p_cx.tile([64, 512], F32, name=f"cx{h0_}{g0}{c}",
                               tag="cx")
                for i in range(GRP):
                    nc.tensor.matmul(
                        cp[:], vss_g[i],
                        ets_g[i][c // 2][:, (c % 2) * 512:(c % 2) * 512 + 512],
                        start=(i == 0), stop=(i == GRP - 1))
                dsl = acc_[:, c * 512:(c + 1) * 512]
                if g0 == 0:
                    nc.vector.tensor_copy(dsl, cp[:])
                else:
                    nc.vector.tensor_add(dsl, dsl, cp[:])

            for h in range(HPC):
                pr, off = divmod(h, 2)
                off *= 64
                kT, qT = qkT["k"][pr], qkT["q"][pr]
                fill = filler.get(h, [])
                acc = p_cc.tile([64, S], F16, name=f"ctxT{h}", tag="cc")
                ets, vss = [], []
                for t in range(ST):
                    pa = p_mm.tile([128, 1024], F32, name=f"ps_s{h}{t}a",
                                   tag="mm")
                    pb = p_mm.tile([128, 1024], F32, name=f"ps_s{h}{t}b",
                                   tag="mm")
                    lhsT = kT[off:off + 64, t * 128:(t + 1) * 128]
                    for c, (pt, o2) in enumerate(
                            ((pa, 0), (pa, 512), (pb, 0), (pb, 512))):
                        nc.tensor.matmul(
                            pt[:, o2:o2 + 512], lhsT,
                            qT[off:off + 64, c * 512:(c + 1) * 512],
                            start=True, stop=True)
                    if chunk_queue:
                        emit_chunk()
                    zp = p_z.tile([128, 2], F32, name=f"zp{h}{t}", tag="zp",
                                  bufs=4)
                    et0 = p_et.tile([128, 1024], BF16, name=f"et{h}{t}a",
                                    tag="et")
                    et1 = p_et.tile([128, 1024], BF16, name=f"et{h}{t}b",
                                    tag="et")
                    nc.scalar.activation(et0[:], pa[:], EXP, scale=0.125,
                                         accum_out=zp[:, 0:1])
                    nc.scalar.activation(et1[:], pb[:], EXP, scale=0.125,
                                         accum_out=zp[:, 1:2])
                    if fill:
                        fill.pop(0)()
                    else:
                        dummy_unit()
                    z = p_z.tile([128, 1], F32, name=f"z{h}{t}", tag="z",
                                 bufs=4)
                    nc.vector.reduce_sum(z[:], zp[:], axis=mybir.AxisListType.X)
                    zr = p_z.tile([128, 1], F32, name=f"zr{h}{t}", tag="zr",
                                  bufs=4)
                    nc.vector.reciprocal(zr[:], z[:])
                    vs = p_z.tile([128, 64], BF16, name=f"vs{h}{t}",
                                  tag="vs", bufs=12)
                    nc.vector.tensor_scalar_mul(
                        vs[:], v_t[t][:, h * 64:(h + 1) * 64], zr[:])
                    ets.append((et0, et1))
                    vss.append(vs)
                    if t % GRP == GRP - 1:
                        g0 = t - (GRP - 1)
                        for c in range(SC):
                            chunk_queue.append(
                                (h, g0, ets[g0:t + 1], vss[g0:t + 1], acc, c))
                while fill:
                    fill.pop(0)()
                ctxT.append(acc)
            while chunk_queue:
                emit_chunk()

            # ---- output projection ----
            # out[s, n] = sum_h sum_p ctxT_h[p, s] * lwT_h[p, n]
            for st in range(ST):
                ob = p_ob.tile([128, NUM_OUT], F32, name=f"ob{st}", tag="ob")
                for ncn in range(NC_CH):
                    pool, tg = (p_pf, "pf") if ncn == 0 else (p_cx, "cx")
                    po = pool.tile([128, 512], F32, name=f"ps_o{st}{ncn}",
                                   tag=tg)
                    for hh in range(HPC):
                        nc.tensor.matmul(
                            po[:],
                            ctxT[hh][:, st * 128:(st + 1) * 128],
                            lw_t[hh][:, ncn * 512:(ncn + 1) * 512],
                            start=(hh == 0), stop=(hh == HPC - 1))
                    nc.vector.tensor_copy(
                        ob[:, ncn * 512:(ncn + 1) * 512], po[:])
                nc.sync.dma_start(out_d[st * 128:(st + 1) * 128, :], ob[:])

    nc.compile()
    return nc


_NC_CACHE = None


def _get_nc():
    global _NC_CACHE
    if _NC_CACHE is None:
        _NC_CACHE = build_nc()
    return _NC_CACHE


def _prep_in_maps(x, q_w, q_b, k_w, k_b, v_w, v_b, l_w):
    """Host-side sharding: per-core input dict (core = b*4 + g)."""
    f16 = np.float16
    in_maps = []
    xts = [np.ascontiguousarray(x[b].T.astype(f16)) for b in range(B)]
    ones = np.ones((1, 128), dtype=f16)
    for b in range(B):
        for g in range(4):
            hs = slice(g * HPC, (g + 1) * HPC)
            f0, f1 = g * HPC * P, (g + 1) * HPC * P
            in_maps.append({
                "xt": xts[b],
                "qwT": np.ascontiguousarray(
                    q_w[hs].transpose(2, 0, 1).reshape(D, HPC * P)
                    .astype(f16)),
                "kwT": np.ascontiguousarray(
                    k_w[hs].transpose(2, 0, 1).reshape(D, HPC * P)
                    .astype(f16)),
                "vwT": np.ascontiguousarray(
                    v_w[hs].transpose(2, 0, 1).reshape(D, HPC * P)
                    .astype(f16)),
                "qb": np.ascontiguousarray(q_b[hs].reshape(HPC * P, 1)),
                "kb": np.ascontiguousarray(k_b[hs].reshape(HPC * P, 1)),
                "vb": np.ascontiguousarray(v_b[hs].reshape(1, HPC * P)
                                           .astype(f16)),
                "lwT": np.ascontiguousarray(l_w[:, f0:f1].T.astype(f16)),
                "ones": ones,
            })
    return in_maps


def _run(inputs, trace=False):
    f32 = lambda a: np.asarray(a, dtype=np.float32)
    x = f32(inputs["x"])
    l_b = f32(inputs["l_b"])
    in_maps = _prep_in_maps(
        x, f32(inputs["q_w"]), f32(inputs["q_b"]), f32(inputs["k_w"]),
        f32(inputs["k_b"]), f32(inputs["v_w"]), f32(inputs["v_b"]),
        f32(inputs["l_w"]))
    nc = _get_nc()
    res = run_bass_kernel_spmd(nc, in_maps, list(range(N_CORES)), trace=trace)
    out = np.empty((B, S, NUM_OUT), dtype=np.float32)
    for b in range(B):
        acc = res.results[b * 4]["out"].astype(np.float32)
        for g in range(1, 4):
            acc = acc + res.results[b * 4 + g]["out"]
        out[b] = acc + l_b
    return out, res


def kernel(**inputs):
    out, _ = _run(inputs, trace=False)
    return out


# revision 16
# speedup vs baseline: 1.4956x; 1.0570x over previous
"""Trainium2 Bass kernel for MyMultiAttentionLayer.

Model (reference):
    q = einsum('bsd,hpd->bhsp', x, q_w) + q_b      (same for k, v)
    scores = q @ k^T / sqrt(P)                      [B,H,S,S]
    attn = softmax(scores, axis=2)                  # softmax over the QUERY axis
    ctx = einsum('bhqk,bhkp->bqhp', attn, v)
    out = concat(ctx) @ l_w.T + l_b                 [B,S,NUM_OUT]

Shapes: B=2, S=2048, D=1024, H=16, P=64, NUM_OUT=1024.

Sharding: 8 cores = 2 batches x 4 head-groups (4 heads each).  Each core
computes its batch's attention for its 4 heads plus the partial output
projection over its 256 features; the host sums the 4 partials per batch
(all-reduce equivalent) and adds l_b.

Softmax is over the query axis, so the normalizer Z[k] = sum_q exp(s[q,k])
depends only on k: ctx = sum_k e[q,k]*(v[k,:]/Z[k]) — the normalization is
folded into the 64-wide v rows instead of the 2048-wide attention matrix.

Precision: all matmul inputs are fp16 (11-bit mantissa — same relative
precision as the PE's fast-fp32 "float32r" mode, but with pipelined
weight loads instead of a serial per-matmul reload) accumulated in fp32
PSUM; exp outputs in bf16 (exp range overflows fp16).

Schedule (the PE executes its stream in order, and the HAM clock gate
halves the PE clock unless the PE stays continuously busy):
  * pair-0 q/k projections run first (DMA-paced),
  * scores for one ki-tile are 4 matmuls into one [128,2048] PSUM tile;
    ONE wide exp ACTIVATE (with fused Z accumulate) consumes it —
    (N+352)/1.2ns makes this the cheapest exp schedule at ~2.3us/tile,
  * ctx accumulates in PSUM over groups of 4 ki-tiles; exactly one ctx
    q-chunk (4 matmuls + vector-engine drain into an SBUF accumulator)
    is emitted per iteration, between scores and exp, so et buffers
    recycle steadily and the PE stream is uniform,
  * v projection and pair-1 q/k projections are chopped into transient-
    PSUM units and interleaved as PE filler in heads 0/1; iterations with
    no real filler emit two dummy matmuls to keep the PE saturated (an
    idle PE gets clock-gated to 1.2 GHz, doubling every matmul).

Per-core layouts (transposes are done host-side when staging inputs):
  xt  [D,S]   = x[b].T  fp16              (contraction dim d on partitions)
  qwT [D,4P]  (d, (h,p)) fp16             kwT same, vwT same
  qb  [4P,1]  kb [4P,1]  fp32, vb [1,4P] fp16
  lwT [4P,NUM_OUT] = l_w[:, feat_slice].T fp16
  out [S,NUM_OUT] fp32 partial (no l_b)
"""

import numpy as np

import concourse.bass as bass
import concourse.tile as tile
from concourse import bacc, mybir
from concourse.bass_utils import run_bass_kernel_spmd

B, S, D = 2, 2048, 1024
H, P = 16, 64
NUM_OUT = 1024
N_CORES = 8
HPC = 4                 # heads per core
PAIRS = 2               # head pairs per core (2 heads x 64 = 128 partitions)
DT = D // 128           # 8 d-tiles
ST = S // 128           # 16 s-tiles
SC = S // 512           # 4 s-chunks of 512
NC_CH = NUM_OUT // 512  # 2 output chunks
GRP = 4                 # ki-tiles per ctx PSUM accumulation group

F32 = mybir.dt.float32
F16 = mybir.dt.float16
BF16 = mybir.dt.bfloat16
EXP = mybir.ActivationFunctionType.Exp


def build_nc():
    nc = bacc.Bacc("TRN2", target_bir_lowering=False, debug=False,
                   num_devices=N_CORES)

    xt_d = nc.dram_tensor("xt", [D, S], F16, kind="ExternalInput")
    qwT_d = nc.dram_tensor("qwT", [D, HPC * P], F16, kind="ExternalInput")
    kwT_d = nc.dram_tensor("kwT", [D, HPC * P], F16, kind="ExternalInput")
    vwT_d = nc.dram_tensor("vwT", [D, HPC * P], F16, kind="ExternalInput")
    qb_d = nc.dram_tensor("qb", [HPC * P, 1], F32, kind="ExternalInput")
    kb_d = nc.dram_tensor("kb", [HPC * P, 1], F32, kind="ExternalInput")
    vb_d = nc.dram_tensor("vb", [1, HPC * P], F16, kind="ExternalInput")
    lwT_d = nc.dram_tensor("lwT", [HPC * P, NUM_OUT], F16, kind="ExternalInput")
    ones_d = nc.dram_tensor("ones", [1, 128], F16, kind="ExternalInput")
    out_d = nc.dram_tensor("out", [S, NUM_OUT], F32, kind="ExternalOutput")

    with tile.TileContext(nc) as tc:
        with (
            tc.tile_pool(name="qk", bufs=4) as p_qk,
            tc.tile_pool(name="vv", bufs=ST) as p_v,
            tc.tile_pool(name="cst", bufs=1) as p_c,
            tc.tile_pool(name="zz", bufs=6) as p_z,
            tc.tile_pool(name="et", bufs=10) as p_et,
            tc.tile_pool(name="cc", bufs=HPC) as p_cc,
            tc.tile_pool(name="ob", bufs=2) as p_ob,
            tc.tile_pool(name="xt", bufs=DT) as p_xt,
            tc.tile_pool(name="wst", bufs=3 * DT) as p_w,
            tc.tile_pool(name="mm", bufs=2, space=bass.MemorySpace.PSUM) as p_mm,
            tc.tile_pool(name="cx", bufs=2, space=bass.MemorySpace.PSUM) as p_cx,
            tc.tile_pool(name="pf", bufs=2, space=bass.MemorySpace.PSUM) as p_pf,
        ):
            # ---- stage inputs, in the order the PE needs them ----
            xt, wq, wk, wv = [], [], [], []
            for d in range(DT):
                t = p_w.tile([128, HPC * P], F16, name=f"qw{d}", tag="w")
                nc.sync.dma_start(t[:], qwT_d[d * 128:(d + 1) * 128, :])
                wq.append(t)
                t = p_w.tile([128, HPC * P], F16, name=f"kw{d}", tag="w")
                nc.sync.dma_start(t[:], kwT_d[d * 128:(d + 1) * 128, :])
                wk.append(t)
                t = p_xt.tile([128, S], F16, name=f"xt{d}", tag="xt")
                nc.sync.dma_start(t[:], xt_d[d * 128:(d + 1) * 128, :])
                xt.append(t)
            qb_t, kb_t = [], []
            for pr in range(PAIRS):
                t = p_c.tile([128, 1], F32, name=f"qb{pr}", tag=f"qb{pr}")
                nc.sync.dma_start(t[:], qb_d[pr * 128:(pr + 1) * 128, :])
                qb_t.append(t)
                t = p_c.tile([128, 1], F32, name=f"kb{pr}", tag=f"kb{pr}")
                nc.sync.dma_start(t[:], kb_d[pr * 128:(pr + 1) * 128, :])
                kb_t.append(t)
            vb_t = p_c.tile([1, HPC * P], F16, name="vb", tag="vb")
            nc.sync.dma_start(vb_t[:], vb_d[:, :])
            ones = p_c.tile([1, 128], F16, name="ones", tag="ones")
            nc.sync.dma_start(ones[:], ones_d[:, :])
            for d in range(DT):
                t = p_w.tile([128, HPC * P], F16, name=f"vw{d}", tag="w")
                nc.sync.dma_start(t[:], vwT_d[d * 128:(d + 1) * 128, :])
                wv.append(t)
            lw_t = []
            for h in range(HPC):
                t = p_c.tile([64, NUM_OUT], F16, name=f"lw{h}", tag=f"lw{h}")
                nc.sync.dma_start(t[:], lwT_d[h * 64:(h + 1) * 64, :])
                lw_t.append(t)

            # SBUF destinations for the projections
            qkT = {"q": [], "k": []}
            for nm in ("q", "k"):
                for pr in range(PAIRS):
                    qkT[nm].append(p_qk.tile([128, S], F16,
                                             name=f"{nm}T{pr}", tag="qk"))
            v_t = [p_v.tile([128, HPC * P], F16, name=f"v{st}", tag="v")
                   for st in range(ST)]

            # ---- projection work units (transient PSUM, usable as filler)
            def qk_unit(nm, pr, c):
                # qT/kT[p_hp, s] = sum_d wT[d, p_hp] * xt[d, s] for one
                # 512-wide s-chunk
                wts = wq if nm == "q" else wk
                bias = qb_t if nm == "q" else kb_t
                ps = p_pf.tile([128, 512], F32, name=f"pp_{nm}{pr}{c}",
                               tag="pf")
                for d in range(DT):
                    nc.tensor.matmul(
                        ps[:], wts[d][:, pr * 128:(pr + 1) * 128],
                        xt[d][:, c * 512:(c + 1) * 512],
                        start=(d == 0), stop=(d == DT - 1))
                nc.vector.tensor_scalar_add(
                    qkT[nm][pr][:, c * 512:(c + 1) * 512], ps[:],
                    bias[pr][:])

            def v_unit(st):
                # v[s, hp] = sum_d xt[d, s] * vwT[d, hp]  (+ ones^T @ vb)
                ps = p_pf.tile([128, 512], F32, name=f"pp_v{st}", tag="pf")
                for d in range(DT):
                    nc.tensor.matmul(
                        ps[:, :HPC * P],
                        xt[d][:, st * 128:(st + 1) * 128], wv[d][:],
                        start=(d == 0), stop=False)
                nc.tensor.matmul(ps[:, :HPC * P], ones[:], vb_t[:],
                                 start=False, stop=True)
                nc.vector.tensor_copy(v_t[st][:], ps[:, :HPC * P])

            ndum = [0]

            def dummy_unit(n=2):
                # keep-warm matmuls: the HAM activity monitor halves the PE
                # clock whenever the PE idles; burning ~0.5us here is far
                # cheaper than running every later matmul at 1.2 GHz.
                ps = p_pf.tile([128, 512], F32, name=f"pp_d{ndum[0]}",
                               tag="pf")
                ndum[0] += 1
                for i in range(n):
                    nc.tensor.matmul(ps[:], xt[0][:, :128], xt[0][:, :512],
                                     start=(i == 0), stop=(i == n - 1))

            # pair-0 q/k first, d-outer so each matmul issues as soon as
            # its xt d-tile lands (a chunk-at-a-time unit would hold a PSUM
            # slot while waiting for the last xt DMA): attention on head 0
            # starts right after the input DMAs land.
            for nm in ("q", "k"):
                wts = wq if nm == "q" else wk
                bias = qb_t if nm == "q" else kb_t
                ps2 = [p_mm.tile([128, 1024], F32, name=f"pp0_{nm}{i}",
                                 tag="mm") for i in range(2)]
                for d in range(DT):
                    lhsT = wts[d][:, 0:128]
                    for c in range(SC):
                        nc.tensor.matmul(
                            ps2[c // 2][:, (c % 2) * 512:(c % 2) * 512 + 512],
                            lhsT, xt[d][:, c * 512:(c + 1) * 512],
                            start=(d == 0), stop=(d == DT - 1))
                for c in range(SC):
                    nc.vector.tensor_scalar_add(
                        qkT[nm][0][:, c * 512:(c + 1) * 512],
                        ps2[c // 2][:, (c % 2) * 512:(c % 2) * 512 + 512],
                        bias[0][:])

            # pair-0 partial of the output projection, computed as PE
            # filler during heads 2/3 (ctxT[0..1] are complete by then);
            # the tail only adds the pair-1 half.
            o01 = [p_ob.tile([128, NUM_OUT], F32, name=f"o01_{st}",
                             tag="o01", bufs=ST) for st in range(ST)]

            def out01_unit(st):
                for ncn in range(NC_CH):
                    po = p_pf.tile([128, 512], F32, name=f"po0_{st}{ncn}",
                                   tag="pf")
                    for hh in range(2):
                        nc.tensor.matmul(
                            po[:],
                            ctxT[hh][:, st * 128:(st + 1) * 128],
                            lw_t[hh][:, ncn * 512:(ncn + 1) * 512],
                            start=(hh == 0), stop=(hh == 1))
                    nc.vector.tensor_copy(
                        o01[st][:, ncn * 512:(ncn + 1) * 512], po[:])

            filler = {0: [lambda st=st: v_unit(st) for st in range(ST)],
                      1: [lambda nm=nm, c=c: qk_unit(nm, 1, c)
                          for nm in ("q", "k") for c in range(SC)],
                      2: [lambda st=st: out01_unit(st) for st in range(ST)]}

            # ---- attention: uniform per-iteration emission ----
            # scoresT[k_i, q_i] = sum_p kT[p, k_i] * qT[p, q_i]   (K=64)
            # one wide exp per ki-tile: eT = exp(scoresT/8), Z fused
            ctxT = []
            chunk_queue = []

            def emit_chunk():
                h0_, g0, ets_g, vss_g, acc_, c = chunk_queue.pop(0)
                cp = p_cx.tile([64, 512], F32, name=f"cx{h0_}{g0}{c}",
                               tag="cx")
                for i in range(GRP):
                    nc.tensor.matmul(
                        cp[:], vss_g[i],
                        ets_g[i][c // 2][:, (c % 2) * 512:(c % 2) * 512 + 512],
                        start=(i == 0), stop=(i == GRP - 1))
                dsl = acc_[:, c * 512:(c + 1) * 512]
                if g0 == 0:
                    nc.vector.tensor_copy(dsl, cp[:])
                else:
                    nc.vector.tensor_add(dsl, dsl, cp[:])

            for h in range(HPC):
                pr, off = divmod(h, 2)
                off *= 64
                kT, qT = qkT["k"][pr], qkT["q"][pr]
                fill = filler.get(h, [])
                acc = p_cc.tile([64, S], F16, name=f"ctxT{h}", tag="cc")
                ets, vss = [], []
                for t in range(ST):
                    pa = p_mm.tile([128, 1024], F32, name=f"ps_s{h}{t}a",
                                   tag="mm")
                    pb = p_mm.tile([128, 1024], F32, name=f"ps_s{h}{t}b",
                                   tag="mm")
                    lhsT = kT[off:off + 64, t * 128:(t + 1) * 128]
                    for c, (pt, o2) in enumerate(
                            ((pa, 0), (pa, 512), (pb, 0), (pb, 512))):
                        nc.tensor.matmul(
                            pt[:, o2:o2 + 512], lhsT,
                            qT[off:off + 64, c * 512:(c + 1) * 512],
                            start=True, stop=True)
                    if chunk_queue:
                        emit_chunk()
                    zp = p_z.tile([128, 2], F32, name=f"zp{h}{t}", tag="zp",
                                  bufs=4)
                    et0 = p_et.tile([128, 1024], BF16, name=f"et{h}{t}a",
                                    tag="et")
                    et1 = p_et.tile([128, 1024], BF16, name=f"et{h}{t}b",
                                    tag="et")
                    nc.scalar.activation(et0[:], pa[:], EXP, scale=0.125,
                                         accum_out=zp[:, 0:1])
                    nc.scalar.activation(et1[:], pb[:], EXP, scale=0.125,
                                         accum_out=zp[:, 1:2])
                    if fill:
                        fill.pop(0)()
                    else:
                        dummy_unit()
                    z = p_z.tile([128, 1], F32, name=f"z{h}{t}", tag="z",
                                 bufs=4)
                    nc.vector.reduce_sum(z[:], zp[:], axis=mybir.AxisListType.X)
                    zr = p_z.tile([128, 1], F32, name=f"zr{h}{t}", tag="zr",
                                  bufs=4)
                    nc.vector.reciprocal(zr[:], z[:])
                    vs = p_z.tile([128, 64], BF16, name=f"vs{h}{t}",
                                  tag="vs", bufs=12)
                    nc.vector.tensor_scalar_mul(
                        vs[:], v_t[t][:, h * 64:(h + 1) * 64], zr[:])
                    ets.append((et0, et1))
                    vss.append(vs)
                    if t % GRP == GRP - 1:
                        g0 = t - (GRP - 1)
                        for c in range(SC):
                            chunk_queue.append(
                                (h, g0, ets[g0:t + 1], vss[g0:t + 1], acc, c))
                while fill:
                    fill.pop(0)()
                ctxT.append(acc)
            while chunk_queue:
                emit_chunk()

            # ---- output projection tail: add the pair-1 half ----
            # out[s, n] = o01[s, n] + sum_{h in 2,3} ctxT_h[p,s]*lwT_h[p,n]
            for st in range(ST):
                ob = p_ob.tile([128, NUM_OUT], F32, name=f"ob{st}", tag="ob")
                for ncn in range(NC_CH):
                    pool, tg = (p_pf, "pf") if ncn == 0 else (p_cx, "cx")
                    po = pool.tile([128, 512], F32, name=f"ps_o{st}{ncn}",
                                   tag=tg)
                    for hh in (2, 3):
                        nc.tensor.matmul(
                            po[:],
                            ctxT[hh][:, st * 128:(st + 1) * 128],
                            lw_t[hh][:, ncn * 512:(ncn + 1) * 512],
                            start=(hh == 2), stop=(hh == 3))
                    nc.vector.tensor_add(
                        ob[:, ncn * 512:(ncn + 1) * 512],
                        o01[st][:, ncn * 512:(ncn + 1) * 512], po[:])
                nc.sync.dma_start(out_d[st * 128:(st + 1) * 128, :], ob[:])

    nc.compile()
    return nc


_NC_CACHE = None


def _get_nc():
    global _NC_CACHE
    if _NC_CACHE is None:
        _NC_CACHE = build_nc()
    return _NC_CACHE


def _prep_in_maps(x, q_w, q_b, k_w, k_b, v_w, v_b, l_w):
    """Host-side sharding: per-core input dict (core = b*4 + g)."""
    f16 = np.float16
    in_maps = []
    xts = [np.ascontiguousarray(x[b].T.astype(f16)) for b in range(B)]
    ones = np.ones((1, 128), dtype=f16)
    for b in range(B):
        for g in range(4):
            hs = slice(g * HPC, (g + 1) * HPC)
            f0, f1 = g * HPC * P, (g + 1) * HPC * P
            in_maps.append({
                "xt": xts[b],
                "qwT": np.ascontiguousarray(
                    q_w[hs].transpose(2, 0, 1).reshape(D, HPC * P)
                    .astype(f16)),
                "kwT": np.ascontiguousarray(
                    k_w[hs].transpose(2, 0, 1).reshape(D, HPC * P)
                    .astype(f16)),
                "vwT": np.ascontiguousarray(
                    v_w[hs].transpose(2, 0, 1).reshape(D, HPC * P)
                    .astype(f16)),
                "qb": np.ascontiguousarray(q_b[hs].reshape(HPC * P, 1)),
                "kb": np.ascontiguousarray(k_b[hs].reshape(HPC * P, 1)),
                "vb": np.ascontiguousarray(v_b[hs].reshape(1, HPC * P)
                                           .astype(f16)),
                "lwT": np.ascontiguousarray(l_w[:, f0:f1].T.astype(f16)),
                "ones": ones,
            })
    return in_maps


def _run(inputs, trace=False):
    f32 = lambda a: np.asarray(a, dtype=np.float32)
    x = f32(inputs["x"])
    l_b = f32(inputs["l_b"])
    in_maps = _prep_in_maps(
        x, f32(inputs["q_w"]), f32(inputs["q_b"]), f32(inputs["k_w"]),
        f32(inputs["k_b"]), f32(inputs["v_w"]), f32(inputs["v_b"]),
        f32(inputs["l_w"]))
    nc = _get_nc()
    res = run_bass_kernel_spmd(nc, in_maps, list(range(N_CORES)), trace=trace)
    out = np.empty((B, S, NUM_OUT), dtype=np.float32)
    for b in range(B):
        acc = res.results[b * 4]["out"].astype(np.float32)
        for g in range(1, 4):
            acc = acc + res.results[b * 4 + g]["out"]
        out[b] = acc + l_b
    return out, res


def kernel(**inputs):
    out, _ = _run(inputs, trace=False)
    return out
